# revision 32
# baseline (speedup 1.0000x reference)
"""Trainium2 Bass kernel for DepthwiseSeparableConv.

Reference computation (NCHW, fp32):
    y   = depthwise_conv7x7(x, dw_w, pad=3, groups=C_IN) + dw_b
    out = pointwise_1x1(y, pw_w) + pw_b

Shapes: x [16, 256, 64, 64], dw_w [256,1,7,7], dw_b [256],
        pw_w [512,256,1,1], pw_b [512]  ->  out [16, 512, 64, 64].

Strategy: data-parallel over batch across 8 NeuronCores (2 batches/core).
Inside each core:
  Phase A (depthwise): channels on partitions, pixels on the free dim.
      Implementations:
        - "dve": 49 scalar_tensor_tensor MACs per 128-channel block (simple).
        - "pe":  banded matmuls on the TensorEngine (fast). Per channel pair
                 (2 channels x 64 H rows = 128 partitions) and kernel column
                 dx, a host-precomputed 64x64 banded matrix applies the conv
                 along H while the W shift comes from clipped column ranges,
                 accumulating all 7 dx into PSUM via has_written semantics.
  Phase B (pointwise): out[o, px] = sum_c pw[c, o] * y[c, px] as dense
      matmuls, contraction over channels (2 blocks of 128), N=512 pixel tiles.
dw_b is folded into an effective pointwise bias on the host (exact, since
the pointwise is linear): pw_b_eff = pw_b + pw_mat.T @ dw_b.
"""

import os
import sys

sys.path.insert(0, "/opt/trn_rl_repo")

import numpy as np

import concourse.bass as bass
import concourse.mybir as mybir
from concourse.tile import TileContext
from concourse.bass_utils import run_bass_kernel_spmd

F32 = mybir.dt.float32

N_CORES = 8
B, C, O, H, W, K, PAD = 16, 256, 512, 64, 64, 7, 3
B_LOC = B // N_CORES          # 2 batches per core
CBLK = C // 128               # 2 channel blocks
OBLK = O // 128               # 4 output-channel blocks
NPAIR = C // 2                # 128 channel pairs (for "pe" depthwise)
PXT = 512                     # pixel-tile (free dim) for pointwise matmuls
HPT = PXT // W                # h-rows per pixel tile (8)

LAST_EXEC_NS = None

# "pe16": bf16 banded depthwise on PE + float32r pointwise (fast)
# "pe":   fp32 banded depthwise on PE + fp32 pointwise
# "dve":  fp32 VectorEngine MAC depthwise (simple reference)
DW_IMPL = os.environ.get("DSC_DW_IMPL", "pe16")


def _clip(lo, hi):
    return max(lo, 0), min(hi, 64)


def _build_dve_phase_a(nc, tc, pools, x_d, dwt_sb, y_tiles, b):
    """Depthwise via 49 per-partition-scalar MACs on DVE, per channel block."""
    xp_pool = pools["xp"]
    for cb in range(CBLK):
        xs = xp_pool.tile([128, H, W], F32)
        nc.default_dma_engine.dma_start(
            out=xs[:], in_=x_d[b, cb * 128 : (cb + 1) * 128, :, :]
        )
        y = y_tiles[cb]
        # center tap first: initializes every output element (full range)
        taps = [(3, 3)] + [
            (dy, dx) for dy in range(K) for dx in range(K) if (dy, dx) != (3, 3)
        ]
        for dy, dx in taps:
            oh0, oh1 = _clip(PAD - dy, H + PAD - dy)
            ow0, ow1 = _clip(PAD - dx, W + PAD - dx)
            ih0, iw0 = oh0 + dy - PAD, ow0 + dx - PAD
            src = xs[:, ih0 : ih0 + (oh1 - oh0), iw0 : iw0 + (ow1 - ow0)]
            dst = y[:, oh0:oh1, ow0:ow1]
            sc = dwt_sb[:, cb, dy * K + dx : dy * K + dx + 1]
            if (dy, dx) == (3, 3):
                nc.vector.tensor_scalar(
                    out=dst, in0=src, scalar1=sc, scalar2=None,
                    op0=mybir.AluOpType.mult,
                )
            else:
                nc.vector.scalar_tensor_tensor(
                    out=dst, in0=src, scalar=sc, in1=dst,
                    op0=mybir.AluOpType.mult, op1=mybir.AluOpType.add,
                )


def _build_pe_phase_a(nc, tc, pools, x_d, bands_d, y_tiles, dt_a=F32):
    """Depthwise via banded matmuls on the TensorEngine.

    Per channel pair j (channels 2j, 2j+1): SBUF tile xp holds
    [partitions = (q in 2, h' in 64), free = (b in B_LOC, wpad in 70)]
    (x is host-padded along W with 3 zeros each side). For dx in 0..6 and
    q in 0..1, a matmul accumulates the H-direction conv into
    ps[(q,h), (b,w)]:
        lhsT = band block [K=64 (h'), M=64 (h)]  (host-precomputed,
               band[h',h] = dw[c, h'-h+3, dx], zero outside the 7-diag)
        rhs  = xp[q rows, :, dx:dx+64]          (W shift via AP offset)
    The 14 matmuls accumulate via PSUM has_written semantics (start=True
    only on the very first, which clears the whole bank). The result pair
    tile is copied PSUM->SBUF (DVE) and DMA-scattered into the
    channel-major y tiles [c, h, b, w].
    """
    xp_pool = pools["xp_pe"]
    band_pool = pools["band"]
    psA_pool = pools["psA"]
    yq_pool = pools["yq"]
    WP = W + 2 * PAD
    for j in range(NPAIR):
        cb, c_lo = (2 * j) // 128, (2 * j) % 128
        bt = band_pool.tile([128, K, 64], dt_a, name="bt")
        nc.default_dma_engine.dma_start(out=bt[:], in_=bands_d[j])
        xp = xp_pool.tile([128, B_LOC, WP], dt_a, name="xp")
        for b in range(B_LOC):
            nc.default_dma_engine.dma_start(
                out=xp[:, b, :], in_=x_d[b, 2 * j : 2 * j + 2, :, :]
            )
        ps = psA_pool.tile([128, B_LOC, W], F32, name="psA")
        for dx in range(K):
            for q in range(2):
                # per-q accumulation groups: the pending-zero clear of a
                # start=True matmul covers only the partitions it writes
                nc.tensor.matmul(
                    ps[q * 64 : (q + 1) * 64, :, :],
                    bt[q * 64 : (q + 1) * 64, dx, :],
                    xp[q * 64 : (q + 1) * 64, :, dx : dx + W],
                    start=(dx == 0),
                    stop=(dx == K - 1),
                )
        yq = yq_pool.tile([128, B_LOC, W], F32, name="yq")
        nc.vector.tensor_copy(out=yq[:], in_=ps[:])
        for q in range(2):
            nc.default_dma_engine.dma_start(
                out=y_tiles[cb][c_lo + q : c_lo + q + 1, :, :, :],
                in_=yq[q * 64 : (q + 1) * 64, :, :],
            )


GRP_PAIRS = 16  # band-load group size for v4
GRP_PAIRS5 = 8  # band-load group size for v5 (padded bands, 2x bytes)
SPLIT_SCATTER = os.environ.get("DSC_SPLIT_SCATTER", "0") == "1"


def build_nc_v5(reps=1):
    """v5: like v4 but with padded block-diagonal [128x128] bands.

    One matmul per (pair, dx) — K=128 contraction covers both channels of
    the pair at once (the off-diagonal quadrants are zero), halving the
    PE instruction count and enabling FWL bf16 weight loads. y scatters
    alternate between the two HWDGE queues (sync / scalar).
    """
    nc = bass.Bass()
    BF16 = mybir.dt.bfloat16
    F32R = mybir.dt.float32r
    WP = W + 2 * PAD
    G = GRP_PAIRS5
    x_d = nc.dram_tensor("x", [128, NPAIR, B_LOC, WP], BF16, kind="ExternalInput")
    bands_d = nc.dram_tensor(
        "bands", [128, NPAIR, K, 128], BF16, kind="ExternalInput"
    )
    pw_d = nc.dram_tensor("pw", [CBLK, 128, O], F32R, kind="ExternalInput")
    pwb_d = nc.dram_tensor("pwb", [128, OBLK], F32, kind="ExternalInput")
    out_d = nc.dram_tensor("out", [B_LOC, O, H, W], F32, kind="ExternalOutput")

    with TileContext(nc) as tc:
        with (
            tc.tile_pool(name="consts", bufs=1) as consts,
            tc.tile_pool(name="band", bufs=2) as band_pool,
            tc.tile_pool(name="y", bufs=1) as y_pool,
            tc.tile_pool(name="yq", bufs=6) as yq_pool,
            tc.tile_pool(name="ob", bufs=2) as ob_pool,
            tc.tile_pool(name="psA", bufs=4, space="PSUM") as psA_pool,
            tc.tile_pool(name="psB", bufs=2, space="PSUM") as psB_pool,
        ):
            pw_sb = consts.tile([128, CBLK, O], F32R)
            for cb in range(CBLK):
                nc.sync.dma_start(out=pw_sb[:, cb, :], in_=pw_d[cb])
            pwb_sb = consts.tile([128, OBLK], F32)
            nc.sync.dma_start(out=pwb_sb[:], in_=pwb_d[:])

            y_tiles = [
                y_pool.tile([128, H, B_LOC, W], F32, name=f"y{cb}")
                for cb in range(CBLK)
            ]

            for rep in range(reps):
                x_sb = consts.tile(
                    [128, NPAIR, B_LOC, WP], BF16, name=f"x_sb{rep}", tag="x_sb"
                )
                nc.sync.dma_start(out=x_sb[:], in_=x_d[:])

                # ---- Phase A ----
                for g in range(NPAIR // G):
                    bt = band_pool.tile([128, G, K, 128], BF16, name="bt")
                    nc.sync.dma_start(
                        out=bt[:], in_=bands_d[:, g * G : (g + 1) * G]
                    )
                    for jj in range(G):
                        j = g * G + jj
                        cb, c_lo = (2 * j) // 128, (2 * j) % 128
                        ps = psA_pool.tile([128, B_LOC, W], F32, name="psA")
                        for dx in range(K):
                            nc.tensor.matmul(
                                ps[:],
                                bt[:, jj, dx, :],
                                x_sb[:, j, :, dx : dx + W],
                                start=(dx == 0),
                                stop=(dx == K - 1),
                            )
                        yq = yq_pool.tile([128, B_LOC, W], F32, name="yq")
                        nc.vector.tensor_copy(out=yq[:], in_=ps[:])
                        eng = nc.scalar if (j % 2 == 0) else nc.sync
                        eng.dma_start(
                            out=y_tiles[cb][c_lo : c_lo + 2, :, :, :],
                            in_=yq[:],
                        )

                # ---- Phase B: pointwise (fp32r) ----
                GRPB = 2
                for b in range(B_LOC):
                    for ob in range(OBLK):
                        osb = ob_pool.tile([128, H, W], F32, name="osb")
                        for g2 in range(H // HPT // GRPB):
                            ps = psB_pool.tile([128, GRPB, HPT, W], F32, name="psB")
                            for cb in range(CBLK):
                                for i in range(GRPB):
                                    pt = g2 * GRPB + i
                                    rhs = y_tiles[cb][
                                        :, pt * HPT : (pt + 1) * HPT, b, :
                                    ].bitcast(F32R)
                                    nc.tensor.matmul(
                                        ps[:, i, :, :],
                                        pw_sb[:, cb, ob * 128 : (ob + 1) * 128],
                                        rhs,
                                        start=(cb == 0),
                                        stop=(cb == CBLK - 1),
                                    )
                            for i in range(GRPB):
                                pt = g2 * GRPB + i
                                nc.scalar.add(
                                    osb[:, pt * HPT : (pt + 1) * HPT, :],
                                    ps[:, i, :, :],
                                    pwb_sb[:, ob : ob + 1],
                                )
                        nc.sync.dma_start(
                            out=out_d[b, ob * 128 : (ob + 1) * 128, :, :],
                            in_=osb[:],
                        )
    return nc


def build_nc_v4(reps=1):
    """v4: bf16 banded depthwise + fp32r pointwise, DMA-count-optimized.

    - x pre-shuffled on host to [128=(q,h'), NPAIR, B_LOC, WP] bf16:
      ONE resident SBUF load; matmul rhs slices directly from it.
    - bands pre-shuffled to [128, NPAIR, K, 64] bf16, loaded in groups.
    - y scatter: one SBUF->SBUF DMA per pair (both channels), issued on the
      scalar-engine HWDGE queue to parallelize with sync-queue DMAs.
    - output: staged per (b, oblk) into a [128, H, W] tile, single DMA out.
    """
    nc = bass.Bass()
    BF16 = mybir.dt.bfloat16
    F32R = mybir.dt.float32r
    WP = W + 2 * PAD
    x_d = nc.dram_tensor("x", [128, NPAIR, B_LOC, WP], BF16, kind="ExternalInput")
    bands_d = nc.dram_tensor("bands", [128, NPAIR, K, 64], BF16, kind="ExternalInput")
    pw_d = nc.dram_tensor("pw", [CBLK, 128, O], F32R, kind="ExternalInput")
    pwb_d = nc.dram_tensor("pwb", [128, OBLK], F32, kind="ExternalInput")
    out_d = nc.dram_tensor("out", [B_LOC, O, H, W], F32, kind="ExternalOutput")

    with TileContext(nc) as tc:
        with (
            tc.tile_pool(name="consts", bufs=1) as consts,
            tc.tile_pool(name="band", bufs=2) as band_pool,
            tc.tile_pool(name="y", bufs=1) as y_pool,
            tc.tile_pool(name="yq", bufs=6) as yq_pool,
            tc.tile_pool(name="ob", bufs=2) as ob_pool,
            tc.tile_pool(name="psA", bufs=4, space="PSUM") as psA_pool,
            tc.tile_pool(name="psB", bufs=2, space="PSUM") as psB_pool,
        ):
            pw_sb = consts.tile([128, CBLK, O], F32R)
            for cb in range(CBLK):
                nc.sync.dma_start(out=pw_sb[:, cb, :], in_=pw_d[cb])
            pwb_sb = consts.tile([128, OBLK], F32)
            nc.sync.dma_start(out=pwb_sb[:], in_=pwb_d[:])

            y_tiles = [
                y_pool.tile([128, H, B_LOC, W], F32, name=f"y{cb}")
                for cb in range(CBLK)
            ]

            for _rep in range(reps):
                _build_v4_body(
                    nc, tc, consts, band_pool, yq_pool, ob_pool,
                    psA_pool, psB_pool,
                    x_d, bands_d, out_d, pw_sb, pwb_sb, y_tiles, _rep,
                )
    return nc


def _build_v4_body(
    nc, tc, consts, band_pool, yq_pool, ob_pool, psA_pool, psB_pool,
    x_d, bands_d, out_d, pw_sb, pwb_sb, y_tiles, rep,
):
    BF16 = mybir.dt.bfloat16
    F32R = mybir.dt.float32r
    WP = W + 2 * PAD
    if True:
        if True:
            x_sb = consts.tile(
                [128, NPAIR, B_LOC, WP], BF16, name=f"x_sb{rep}", tag="x_sb"
            )
            nc.sync.dma_start(out=x_sb[:], in_=x_d[:])

            # ---- Phase A: banded depthwise on PE ----
            for g in range(NPAIR // GRP_PAIRS):
                bt = band_pool.tile([128, GRP_PAIRS, K, 64], BF16, name="bt")
                nc.sync.dma_start(
                    out=bt[:], in_=bands_d[:, g * GRP_PAIRS : (g + 1) * GRP_PAIRS]
                )
                for jj in range(GRP_PAIRS):
                    j = g * GRP_PAIRS + jj
                    cb, c_lo = (2 * j) // 128, (2 * j) % 128
                    ps = psA_pool.tile([128, B_LOC, W], F32, name="psA")
                    for dx in range(K):
                        for q in range(2):
                            nc.tensor.matmul(
                                ps[q * 64 : (q + 1) * 64, :, :],
                                bt[q * 64 : (q + 1) * 64, jj, dx, :],
                                x_sb[q * 64 : (q + 1) * 64, j, :, dx : dx + W],
                                start=(dx == 0),
                                stop=(dx == K - 1),
                            )
                    yq = yq_pool.tile([128, B_LOC, W], F32, name="yq")
                    nc.vector.tensor_copy(out=yq[:], in_=ps[:])
                    # single scatter: channels (2j, 2j+1) are adjacent partitions
                    eng = nc.scalar if (SPLIT_SCATTER and j % 2 == 0) else (
                        nc.sync if SPLIT_SCATTER else nc.scalar
                    )
                    eng.dma_start(
                        out=y_tiles[cb][c_lo : c_lo + 2, :, :, :],
                        in_=yq[:],
                    )

            # ---- Phase B: pointwise (fp32r) ----
            GRPB = 2
            for b in range(B_LOC):
                for ob in range(OBLK):
                    osb = ob_pool.tile([128, H, W], F32, name="osb")
                    for g in range(H // HPT // GRPB):
                        ps = psB_pool.tile([128, GRPB, HPT, W], F32, name="psB")
                        for cb in range(CBLK):
                            for i in range(GRPB):
                                pt = g * GRPB + i
                                rhs = y_tiles[cb][
                                    :, pt * HPT : (pt + 1) * HPT, b, :
                                ].bitcast(F32R)
                                nc.tensor.matmul(
                                    ps[:, i, :, :],
                                    pw_sb[:, cb, ob * 128 : (ob + 1) * 128],
                                    rhs,
                                    start=(cb == 0),
                                    stop=(cb == CBLK - 1),
                                )
                        for i in range(GRPB):
                            pt = g * GRPB + i
                            nc.scalar.add(
                                osb[:, pt * HPT : (pt + 1) * HPT, :],
                                ps[:, i, :, :],
                                pwb_sb[:, ob : ob + 1],
                            )
                    nc.sync.dma_start(
                        out=out_d[b, ob * 128 : (ob + 1) * 128, :, :],
                        in_=osb[:],
                    )
    return nc


def build_nc(dw_impl=DW_IMPL):
    reps = int(dw_impl.split("@")[1]) if "@" in dw_impl else 1
    if dw_impl.startswith("v4"):
        return build_nc_v4(reps=reps)
    if dw_impl.startswith("v5"):
        return build_nc_v5(reps=reps)
    nc = bass.Bass()
    BF16 = mybir.dt.bfloat16
    F32R = mybir.dt.float32r
    dt_a = BF16 if dw_impl == "pe16" else F32
    dt_b = F32R if dw_impl == "pe16" else F32
    if dw_impl in ("pe", "pe16"):
        x_d = nc.dram_tensor(
            "x", [B_LOC, C, H, W + 2 * PAD], dt_a, kind="ExternalInput"
        )
        bands_d = nc.dram_tensor(
            "bands", [NPAIR, 128, K, 64], dt_a, kind="ExternalInput"
        )
    else:
        x_d = nc.dram_tensor("x", [B_LOC, C, H, W], F32, kind="ExternalInput")
        dwt_d = nc.dram_tensor("dwt", [128, CBLK, K * K], F32, kind="ExternalInput")
    pw_d = nc.dram_tensor("pw", [CBLK, 128, O], dt_b, kind="ExternalInput")
    pwb_d = nc.dram_tensor("pwb", [128, OBLK], F32, kind="ExternalInput")
    out_d = nc.dram_tensor("out", [B_LOC, O, H, W], F32, kind="ExternalOutput")

    with TileContext(nc) as tc:
        with (
            tc.tile_pool(name="consts", bufs=1) as consts,
            tc.tile_pool(name="xp", bufs=3) as xp_pool,
            tc.tile_pool(name="y", bufs=1) as y_pool,
            tc.tile_pool(name="ob", bufs=3) as ob_pool,
            tc.tile_pool(name="psB", bufs=2, space="PSUM") as psB_pool,
            tc.tile_pool(name="psA", bufs=2, space="PSUM") as psA_pool,
            tc.tile_pool(name="band", bufs=4) as band_pool,
            tc.tile_pool(name="yq", bufs=4) as yq_pool,
        ):
            pools = {
                "xp": xp_pool,
                "xp_pe": xp_pool,
                "band": band_pool,
                "psA": psA_pool,
                "yq": yq_pool,
            }
            pw_sb = consts.tile([128, CBLK, O], dt_b)
            for cb in range(CBLK):
                nc.default_dma_engine.dma_start(out=pw_sb[:, cb, :], in_=pw_d[cb])
            pwb_sb = consts.tile([128, OBLK], F32)
            nc.default_dma_engine.dma_start(out=pwb_sb[:], in_=pwb_d[:])

            if dw_impl in ("pe", "pe16"):
                # y layout: [c 128, h 64, b B_LOC, w 64] per channel block
                y_tiles = [
                    y_pool.tile([128, H, B_LOC, W], F32, name=f"y{cb}")
                    for cb in range(CBLK)
                ]
                _build_pe_phase_a(nc, tc, pools, x_d, bands_d, y_tiles, dt_a=dt_a)
                # Phase B: pointwise, psum groups of 2 pixel-tiles
                GRP = 2
                for b in range(B_LOC):
                    for ob in range(OBLK):
                        for g in range(H // HPT // GRP):
                            ps = psB_pool.tile([128, GRP, HPT, W], F32, name="psB")
                            for cb in range(CBLK):
                                for i in range(GRP):
                                    pt = g * GRP + i
                                    rhs = y_tiles[cb][:, pt * HPT : (pt + 1) * HPT, b, :]
                                    if dt_b != F32:
                                        rhs = rhs.bitcast(dt_b)
                                    nc.tensor.matmul(
                                        ps[:, i, :, :],
                                        pw_sb[:, cb, ob * 128 : (ob + 1) * 128],
                                        rhs,
                                        start=(cb == 0),
                                        stop=(cb == CBLK - 1),
                                    )
                            for i in range(GRP):
                                pt = g * GRP + i
                                osb = ob_pool.tile([128, HPT, W], F32, name="osb")
                                nc.scalar.add(
                                    osb[:], ps[:, i, :, :], pwb_sb[:, ob : ob + 1]
                                )
                                nc.default_dma_engine.dma_start(
                                    out=out_d[b, ob * 128 : (ob + 1) * 128,
                                              pt * HPT : (pt + 1) * HPT, :],
                                    in_=osb[:],
                                )
            else:
                dwt_sb = consts.tile([128, CBLK, K * K], F32)
                nc.default_dma_engine.dma_start(out=dwt_sb[:], in_=dwt_d[:])
                for b in range(B_LOC):
                    y_tiles = [
                        y_pool.tile([128, H, W], F32, tag=f"y{cb}", name=f"y{cb}_{b}")
                        for cb in range(CBLK)
                    ]
                    _build_dve_phase_a(nc, tc, pools, x_d, dwt_sb, y_tiles, b)
                    for ob in range(OBLK):
                        for pt in range(H // HPT):
                            ps = psB_pool.tile([128, HPT, W], F32, name="psB")
                            for cb in range(CBLK):
                                nc.tensor.matmul(
                                    ps[:],
                                    pw_sb[:, cb, ob * 128 : (ob + 1) * 128],
                                    y_tiles[cb][:, pt * HPT : (pt + 1) * HPT, :],
                                    start=(cb == 0),
                                    stop=(cb == CBLK - 1),
                                )
                            osb = ob_pool.tile([128, HPT, W], F32, name="osb")
                            nc.scalar.add(osb[:], ps[:], pwb_sb[:, ob : ob + 1])
                            nc.default_dma_engine.dma_start(
                                out=out_d[b, ob * 128 : (ob + 1) * 128,
                                          pt * HPT : (pt + 1) * HPT, :],
                                in_=osb[:],
                            )
    return nc


def legalize_waits(nc, max_waits=1):
    """This container's walrus accepts only one sync-wait per instruction.

    Hoist extra on_wait conditions into standalone same-engine NoOps placed
    immediately before the instruction (engine programs execute in block
    order, so the waits still complete before the instruction issues).
    """
    n_hoisted = 0
    for f in nc.m.functions:
        for blk in f.blocks:
            insts = list(blk.instructions)
            out = []
            changed = False
            for inst in insts:
                si = inst.sync_info
                if si is not None and si.on_wait and len(si.on_wait) > max_waits:
                    waits = list(si.on_wait)
                    for i, w in enumerate(waits[:-max_waits]):
                        nop = mybir.InstNoOp(name=f"{inst.name}-hw{i}")
                        nop.engine = inst.engine
                        nop.sync_info = mybir.SyncInfo(on_wait=[w], on_update=[])
                        out.append(nop)
                        n_hoisted += 1
                    inst.sync_info = mybir.SyncInfo(
                        on_wait=waits[-max_waits:], on_update=list(si.on_update)
                    )
                    changed = True
                out.append(inst)
            if changed:
                blk.instructions = out
    return n_hoisted


_NC_CACHE = {}


def _get_nc(dw_impl):
    if dw_impl not in _NC_CACHE:
        nc = build_nc(dw_impl)
        legalize_waits(nc)
        _NC_CACHE[dw_impl] = nc
    return _NC_CACHE[dw_impl]


def _build_bands(dw_w):
    """bands[j, q*64+hp, dx, h] = dw_w[2j+q, 0, hp-h+3, dx] (0 outside band)."""
    dw = dw_w[:, 0].reshape(NPAIR, 2, K, K)  # [j, q, dy, dx]
    bands = np.zeros((NPAIR, 2, 64, K, 64), np.float32)
    for dy in range(K):
        for hp in range(64):
            h = hp - dy + PAD
            if 0 <= h < 64:
                bands[:, :, hp, :, h] = dw[:, :, dy, :]
    return np.ascontiguousarray(bands.reshape(NPAIR, 128, K, 64))


def _prep_in_maps(x, dw_w, dw_b, pw_w, pw_b, dw_impl=None):
    """Host-side weight prep + per-core sharding. Returns in_maps list."""
    if dw_impl is None:
        dw_impl = DW_IMPL
    pw_mat = pw_w[:, :, 0, 0].T  # [C, O] (c-major)
    pw = np.ascontiguousarray(pw_mat.reshape(CBLK, 128, O))
    pwb_eff = pw_b + pw_mat.T @ dw_b  # [O]
    pwb = np.ascontiguousarray(pwb_eff.reshape(OBLK, 128).T)  # [128, OBLK]
    if dw_impl.startswith(("v4", "v5")):
        import ml_dtypes

        WP = W + 2 * PAD
        xp = np.zeros((B, C, H, WP), np.float32)
        xp[:, :, :, PAD : PAD + W] = x
        # per-core shard then shuffle to [128=(q,h'), NPAIR, B_LOC, WP]
        # partition p = q*64 + h', where channel c = 2j + q
        bands = _build_bands(dw_w)  # [NPAIR, 128, K, 64]
        if dw_impl.startswith("v5"):
            # padded block-diagonal [128(q,h'), NPAIR, K, 128(q2,h)]
            bp = np.zeros((2, 64, NPAIR, K, 2, 64), np.float32)
            br = bands.reshape(NPAIR, 2, 64, K, 64)
            for q in range(2):
                bp[q, :, :, :, q, :] = br[:, q].transpose(1, 0, 2, 3)
            bands_sh = np.ascontiguousarray(
                bp.reshape(128, NPAIR, K, 128).astype(ml_dtypes.bfloat16)
            )
        else:
            bands_sh = np.ascontiguousarray(
                bands.transpose(1, 0, 2, 3).astype(ml_dtypes.bfloat16)
            )  # [128, NPAIR, K, 64]
        shared = {"bands": bands_sh, "pw": pw, "pwb": pwb}
        in_maps = []
        for k in range(N_CORES):
            xk = xp[k * B_LOC : (k + 1) * B_LOC]  # [B_LOC, C, H, WP]
            # -> [q, h', j, b, wp] -> [(q h'), j, b, wp]
            xr = xk.reshape(B_LOC, NPAIR, 2, H, WP)
            xsh = np.ascontiguousarray(
                xr.transpose(2, 3, 1, 0, 4).reshape(128, NPAIR, B_LOC, WP)
            ).astype(ml_dtypes.bfloat16)
            m = {"x": xsh}
            m.update(shared)
            in_maps.append(m)
        return in_maps
    if dw_impl in ("pe", "pe16"):
        xp = np.zeros((B, C, H, W + 2 * PAD), np.float32)
        xp[:, :, :, PAD : PAD + W] = x
        bands = _build_bands(dw_w)
        if dw_impl == "pe16":
            import ml_dtypes

            xp = xp.astype(ml_dtypes.bfloat16)
            bands = bands.astype(ml_dtypes.bfloat16)
        shared = {"bands": bands, "pw": pw, "pwb": pwb}
        xs = xp
    else:
        dwt = np.ascontiguousarray(
            dw_w[:, 0].reshape(CBLK, 128, K * K).transpose(1, 0, 2)
        )  # [128, CBLK, 49], partition = c_lo
        shared = {"dwt": dwt, "pw": pw, "pwb": pwb}
        xs = x
    in_maps = []
    for k in range(N_CORES):
        m = {"x": np.ascontiguousarray(xs[k * B_LOC : (k + 1) * B_LOC])}
        m.update(shared)
        in_maps.append(m)
    return in_maps


def _make_runner(nc):
    """Compile nc into a pipelined multi-core jitted fn (no donation)."""
    import jax
    from jax.sharding import Mesh, NamedSharding, PartitionSpec
    from jax.experimental.shard_map import shard_map
    from concourse import bass2jax
    from concourse.bass2jax import _bass_exec_p

    bass2jax.install_neuronx_cc_hook()
    n_cores = N_CORES
    partition_name = (
        nc.partition_id_tensor.name if nc.partition_id_tensor else None
    )
    in_names, out_names, out_avals, zero_outs = [], [], [], []
    for alloc in nc.m.functions[0].allocations:
        if not isinstance(alloc, mybir.MemoryLocationSet):
            continue
        name = alloc.memorylocations[0].name
        if alloc.kind == "ExternalInput":
            if name != partition_name:
                in_names.append(name)
        elif alloc.kind == "ExternalOutput":
            out_names.append(name)
            shape = tuple(alloc.tensor_shape)
            dtype = mybir.dt.np(alloc.dtype)
            out_avals.append(jax.core.ShapedArray(shape, dtype))
            zero_outs.append(np.zeros(shape, dtype))
    n_params = len(in_names)
    all_names = in_names + out_names
    if partition_name is not None:
        all_names = all_names + [partition_name]

    def _body(*args):
        operands = list(args)
        if partition_name is not None:
            operands.append(bass2jax.partition_id_tensor())
        outs = _bass_exec_p.bind(
            *operands,
            out_avals=tuple(out_avals),
            in_names=tuple(all_names),
            out_names=tuple(out_names),
            lowering_input_output_aliases=(),
            sim_require_finite=True,
            sim_require_nnan=True,
            nc=nc,
        )
        return tuple(outs)

    devices = jax.devices()[:n_cores]
    mesh = Mesh(np.asarray(devices), ("core",))
    spec = PartitionSpec("core")
    n_all = n_params + len(out_names)
    fn = jax.jit(
        shard_map(
            _body,
            mesh=mesh,
            in_specs=(spec,) * n_all,
            out_specs=(spec,) * len(out_names),
            check_rep=False,
        ),
        keep_unused=True,
    )
    sh = NamedSharding(mesh, spec)
    return fn, in_names, out_names, zero_outs, sh


_FLOOR_CACHE = {}


def _measure_floor(iters):
    """Per-iteration dispatch overhead of a trivial kernel on this session."""
    import time

    import jax

    if "fn" not in _FLOOR_CACHE:
        nc = bass.Bass()
        a_d = nc.dram_tensor("a", [128, 64], F32, kind="ExternalInput")
        o_d = nc.dram_tensor("o", [128, 64], F32, kind="ExternalOutput")
        with TileContext(nc) as tc:
            with tc.tile_pool(name="p", bufs=2) as pool:
                at = pool.tile([128, 64], F32, name="at")
                nc.default_dma_engine.dma_start(out=at[:], in_=a_d[:])
                ot = pool.tile([128, 64], F32, name="ot")
                nc.vector.tensor_copy(out=ot[:], in_=at[:])
                nc.default_dma_engine.dma_start(out=o_d[:], in_=ot[:])
        legalize_waits(nc)
        fn, in_names, out_names, zeros, sh = _make_runner(nc)
        a = jax.device_put(
            np.zeros((N_CORES * 128, 64), np.float32), sh
        )
        z = jax.device_put(np.zeros((N_CORES * 128, 64), np.float32), sh)
        jax.block_until_ready(fn(a, z))
        _FLOOR_CACHE["fn"] = (fn, a, z)
    fn, a, z = _FLOOR_CACHE["fn"]
    t0 = time.perf_counter()
    r = None
    for _ in range(iters):
        r = fn(a, z)
    jax.block_until_ready(r)
    t1 = time.perf_counter()
    return (t1 - t0) / iters


def _bench_impl(impl, in_maps, iters=100):
    """Time one compiled impl; returns (out_arrs_map, raw_per_iter)."""
    import time

    import jax

    nc = _get_nc(impl)
    fn, in_names, out_names, zero_outs, sh = _make_runner(nc)
    concat_in = [
        np.concatenate([np.asarray(in_maps[c][nm]) for c in range(N_CORES)], axis=0)
        for nm in in_names
    ]
    concat_zeros = [
        np.zeros((N_CORES * z.shape[0], *z.shape[1:]), z.dtype) for z in zero_outs
    ]
    dev_in = [jax.device_put(a, sh) for a in concat_in + concat_zeros]
    out_arrs = jax.block_until_ready(fn(*dev_in))
    # time
    best = None
    for _round in range(3):
        t0 = time.perf_counter()
        r = None
        for _ in range(iters):
            r = fn(*dev_in)
        jax.block_until_ready(r)
        t1 = time.perf_counter()
        v = (t1 - t0) / iters
        best = v if best is None else min(best, v)
    out_full = np.asarray(out_arrs[out_names.index("out")])
    out = out_full.reshape(N_CORES, B_LOC, O, H, W).reshape(B, O, H, W)
    return out, best


def bench_reps(x, dw_w, dw_b, pw_w, pw_b, base="v4", reps=5, iters=100):
    """Floor-free timing: (T(reps) - T(1)) / (reps - 1)."""
    in_maps = _prep_in_maps(
        np.ascontiguousarray(np.asarray(x, dtype=np.float32)),
        np.asarray(dw_w, np.float32),
        np.asarray(dw_b, np.float32),
        np.asarray(pw_w, np.float32),
        np.asarray(pw_b, np.float32),
        dw_impl=base,
    )
    out1, t1 = _bench_impl(base, in_maps, iters)
    _, tR = _bench_impl(f"{base}@{reps}", in_maps, iters)
    per_rep = (tR - t1) / (reps - 1)
    return out1, per_rep, t1, tR


def bench(x, dw_w, dw_b, pw_w, pw_b, iters=200):
    """Steady-state timing with floor subtraction.

    Returns (out, marginal_per_iter_s, raw_per_iter_s, floor_s).
    """
    import time

    import jax

    nc = _get_nc(DW_IMPL)
    in_maps = _prep_in_maps(
        np.ascontiguousarray(np.asarray(x, dtype=np.float32)),
        np.asarray(dw_w, np.float32),
        np.asarray(dw_b, np.float32),
        np.asarray(pw_w, np.float32),
        np.asarray(pw_b, np.float32),
    )
    fn, in_names, out_names, zero_outs, sh = _make_runner(nc)
    concat_in = [
        np.concatenate([np.asarray(in_maps[c][nm]) for c in range(N_CORES)], axis=0)
        for nm in in_names
    ]
    concat_zeros = [
        np.zeros((N_CORES * z.shape[0], *z.shape[1:]), z.dtype) for z in zero_outs
    ]
    dev_in = [jax.device_put(a, sh) for a in concat_in + concat_zeros]
    out_arrs = jax.block_until_ready(fn(*dev_in))  # compile + warm

    floor = _measure_floor(iters)
    t0 = time.perf_counter()
    r = None
    for _ in range(iters):
        r = fn(*dev_in)
    jax.block_until_ready(r)
    t1 = time.perf_counter()
    raw = (t1 - t0) / iters
    out_full = np.asarray(out_arrs[out_names.index("out")])
    out = out_full.reshape(N_CORES, B_LOC, O, H, W).reshape(B, O, H, W)
    return out, max(raw - floor, 0.0), raw, floor


def kernel(x, dw_w, dw_b, pw_w, pw_b, trace=False):
    global LAST_EXEC_NS
    in_maps = _prep_in_maps(
        np.ascontiguousarray(np.asarray(x, dtype=np.float32)),
        np.asarray(dw_w, np.float32),
        np.asarray(dw_b, np.float32),
        np.asarray(pw_w, np.float32),
        np.asarray(pw_b, np.float32),
    )
    nc = _get_nc(DW_IMPL)
    res = run_bass_kernel_spmd(nc, in_maps, list(range(N_CORES)), trace=trace)
    LAST_EXEC_NS = res.exec_time_ns
    out = np.concatenate([res.results[k]["out"] for k in range(N_CORES)], axis=0)
    return out


# revision 34
# speedup vs baseline: 1.9385x; 1.9385x over previous
"""Trainium2 Bass kernel for DepthwiseSeparableConv.

Reference computation (NCHW, fp32):
    y   = depthwise_conv7x7(x, dw_w, pad=3, groups=C_IN) + dw_b
    out = pointwise_1x1(y, pw_w) + pw_b

Shapes: x [16, 256, 64, 64], dw_w [256,1,7,7], dw_b [256],
        pw_w [512,256,1,1], pw_b [512]  ->  out [16, 512, 64, 64].

Strategy: data-parallel over batch across 8 NeuronCores (2 batches/core).
Inside each core:
  Phase A (depthwise): channels on partitions, pixels on the free dim.
      Implementations:
        - "dve": 49 scalar_tensor_tensor MACs per 128-channel block (simple).
        - "pe":  banded matmuls on the TensorEngine (fast). Per channel pair
                 (2 channels x 64 H rows = 128 partitions) and kernel column
                 dx, a host-precomputed 64x64 banded matrix applies the conv
                 along H while the W shift comes from clipped column ranges,
                 accumulating all 7 dx into PSUM via has_written semantics.
  Phase B (pointwise): out[o, px] = sum_c pw[c, o] * y[c, px] as dense
      matmuls, contraction over channels (2 blocks of 128), N=512 pixel tiles.
dw_b is folded into an effective pointwise bias on the host (exact, since
the pointwise is linear): pw_b_eff = pw_b + pw_mat.T @ dw_b.
"""

import os
import sys

sys.path.insert(0, "/opt/trn_rl_repo")

import numpy as np

import concourse.bass as bass
import concourse.mybir as mybir
from concourse.tile import TileContext
from concourse.bass_utils import run_bass_kernel_spmd

F32 = mybir.dt.float32

N_CORES = 8
B, C, O, H, W, K, PAD = 16, 256, 512, 64, 64, 7, 3
B_LOC = B // N_CORES          # 2 batches per core
CBLK = C // 128               # 2 channel blocks
OBLK = O // 128               # 4 output-channel blocks
NPAIR = C // 2                # 128 channel pairs (for "pe" depthwise)
PXT = 512                     # pixel-tile (free dim) for pointwise matmuls
HPT = PXT // W                # h-rows per pixel tile (8)

LAST_EXEC_NS = None

# "pe16": bf16 banded depthwise on PE + float32r pointwise (fast)
# "pe":   fp32 banded depthwise on PE + fp32 pointwise
# "dve":  fp32 VectorEngine MAC depthwise (simple reference)
DW_IMPL = os.environ.get("DSC_DW_IMPL", "pe16")


def _clip(lo, hi):
    return max(lo, 0), min(hi, 64)


def _build_dve_phase_a(nc, tc, pools, x_d, dwt_sb, y_tiles, b):
    """Depthwise via 49 per-partition-scalar MACs on DVE, per channel block."""
    xp_pool = pools["xp"]
    for cb in range(CBLK):
        xs = xp_pool.tile([128, H, W], F32)
        nc.default_dma_engine.dma_start(
            out=xs[:], in_=x_d[b, cb * 128 : (cb + 1) * 128, :, :]
        )
        y = y_tiles[cb]
        # center tap first: initializes every output element (full range)
        taps = [(3, 3)] + [
            (dy, dx) for dy in range(K) for dx in range(K) if (dy, dx) != (3, 3)
        ]
        for dy, dx in taps:
            oh0, oh1 = _clip(PAD - dy, H + PAD - dy)
            ow0, ow1 = _clip(PAD - dx, W + PAD - dx)
            ih0, iw0 = oh0 + dy - PAD, ow0 + dx - PAD
            src = xs[:, ih0 : ih0 + (oh1 - oh0), iw0 : iw0 + (ow1 - ow0)]
            dst = y[:, oh0:oh1, ow0:ow1]
            sc = dwt_sb[:, cb, dy * K + dx : dy * K + dx + 1]
            if (dy, dx) == (3, 3):
                nc.vector.tensor_scalar(
                    out=dst, in0=src, scalar1=sc, scalar2=None,
                    op0=mybir.AluOpType.mult,
                )
            else:
                nc.vector.scalar_tensor_tensor(
                    out=dst, in0=src, scalar=sc, in1=dst,
                    op0=mybir.AluOpType.mult, op1=mybir.AluOpType.add,
                )


def _build_pe_phase_a(nc, tc, pools, x_d, bands_d, y_tiles, dt_a=F32):
    """Depthwise via banded matmuls on the TensorEngine.

    Per channel pair j (channels 2j, 2j+1): SBUF tile xp holds
    [partitions = (q in 2, h' in 64), free = (b in B_LOC, wpad in 70)]
    (x is host-padded along W with 3 zeros each side). For dx in 0..6 and
    q in 0..1, a matmul accumulates the H-direction conv into
    ps[(q,h), (b,w)]:
        lhsT = band block [K=64 (h'), M=64 (h)]  (host-precomputed,
               band[h',h] = dw[c, h'-h+3, dx], zero outside the 7-diag)
        rhs  = xp[q rows, :, dx:dx+64]          (W shift via AP offset)
    The 14 matmuls accumulate via PSUM has_written semantics (start=True
    only on the very first, which clears the whole bank). The result pair
    tile is copied PSUM->SBUF (DVE) and DMA-scattered into the
    channel-major y tiles [c, h, b, w].
    """
    xp_pool = pools["xp_pe"]
    band_pool = pools["band"]
    psA_pool = pools["psA"]
    yq_pool = pools["yq"]
    WP = W + 2 * PAD
    for j in range(NPAIR):
        cb, c_lo = (2 * j) // 128, (2 * j) % 128
        bt = band_pool.tile([128, K, 64], dt_a, name="bt")
        nc.default_dma_engine.dma_start(out=bt[:], in_=bands_d[j])
        xp = xp_pool.tile([128, B_LOC, WP], dt_a, name="xp")
        for b in range(B_LOC):
            nc.default_dma_engine.dma_start(
                out=xp[:, b, :], in_=x_d[b, 2 * j : 2 * j + 2, :, :]
            )
        ps = psA_pool.tile([128, B_LOC, W], F32, name="psA")
        for dx in range(K):
            for q in range(2):
                # per-q accumulation groups: the pending-zero clear of a
                # start=True matmul covers only the partitions it writes
                nc.tensor.matmul(
                    ps[q * 64 : (q + 1) * 64, :, :],
                    bt[q * 64 : (q + 1) * 64, dx, :],
                    xp[q * 64 : (q + 1) * 64, :, dx : dx + W],
                    start=(dx == 0),
                    stop=(dx == K - 1),
                )
        yq = yq_pool.tile([128, B_LOC, W], F32, name="yq")
        nc.vector.tensor_copy(out=yq[:], in_=ps[:])
        for q in range(2):
            nc.default_dma_engine.dma_start(
                out=y_tiles[cb][c_lo + q : c_lo + q + 1, :, :, :],
                in_=yq[q * 64 : (q + 1) * 64, :, :],
            )


GRP_PAIRS = 16  # band-load group size for v4
GRP_PAIRS5 = 8  # band-load group size for v5 (padded bands, 2x bytes)
SPLIT_SCATTER = os.environ.get("DSC_SPLIT_SCATTER", "0") == "1"
BAND_BUFS = int(os.environ.get("DSC_BAND_BUFS", "2"))


def build_nc_v5(reps=1):
    """v5: like v4 but with padded block-diagonal [128x128] bands.

    One matmul per (pair, dx) — K=128 contraction covers both channels of
    the pair at once (the off-diagonal quadrants are zero), halving the
    PE instruction count and enabling FWL bf16 weight loads. y scatters
    alternate between the two HWDGE queues (sync / scalar).
    """
    nc = bass.Bass()
    BF16 = mybir.dt.bfloat16
    F32R = mybir.dt.float32r
    WP = W + 2 * PAD
    G = GRP_PAIRS5
    x_d = nc.dram_tensor("x", [128, NPAIR, B_LOC, WP], BF16, kind="ExternalInput")
    bands_d = nc.dram_tensor(
        "bands", [128, NPAIR, K, 128], BF16, kind="ExternalInput"
    )
    pw_d = nc.dram_tensor("pw", [CBLK, 128, O], F32R, kind="ExternalInput")
    pwb_d = nc.dram_tensor("pwb", [128, OBLK], F32, kind="ExternalInput")
    out_d = nc.dram_tensor("out", [B_LOC, O, H, W], F32, kind="ExternalOutput")

    with TileContext(nc) as tc:
        with (
            tc.tile_pool(name="consts", bufs=1) as consts,
            tc.tile_pool(name="band", bufs=2) as band_pool,
            tc.tile_pool(name="y", bufs=1) as y_pool,
            tc.tile_pool(name="yq", bufs=6) as yq_pool,
            tc.tile_pool(name="ob", bufs=2) as ob_pool,
            tc.tile_pool(name="psA", bufs=4, space="PSUM") as psA_pool,
            tc.tile_pool(name="psB", bufs=2, space="PSUM") as psB_pool,
        ):
            pw_sb = consts.tile([128, CBLK, O], F32R)
            for cb in range(CBLK):
                nc.sync.dma_start(out=pw_sb[:, cb, :], in_=pw_d[cb])
            pwb_sb = consts.tile([128, OBLK], F32)
            nc.sync.dma_start(out=pwb_sb[:], in_=pwb_d[:])

            y_tiles = [
                y_pool.tile([128, H, B_LOC, W], F32, name=f"y{cb}")
                for cb in range(CBLK)
            ]

            for rep in range(reps):
                x_sb = consts.tile(
                    [128, NPAIR, B_LOC, WP], BF16, name=f"x_sb{rep}", tag="x_sb"
                )
                nc.sync.dma_start(out=x_sb[:], in_=x_d[:])

                # ---- Phase A ----
                for g in range(NPAIR // G):
                    bt = band_pool.tile([128, G, K, 128], BF16, name="bt")
                    nc.sync.dma_start(
                        out=bt[:], in_=bands_d[:, g * G : (g + 1) * G]
                    )
                    for jj in range(G):
                        j = g * G + jj
                        cb, c_lo = (2 * j) // 128, (2 * j) % 128
                        ps = psA_pool.tile([128, B_LOC, W], F32, name="psA")
                        for dx in range(K):
                            nc.tensor.matmul(
                                ps[:],
                                bt[:, jj, dx, :],
                                x_sb[:, j, :, dx : dx + W],
                                start=(dx == 0),
                                stop=(dx == K - 1),
                            )
                        yq = yq_pool.tile([128, B_LOC, W], F32, name="yq")
                        nc.vector.tensor_copy(out=yq[:], in_=ps[:])
                        eng = nc.scalar if (j % 2 == 0) else nc.sync
                        eng.dma_start(
                            out=y_tiles[cb][c_lo : c_lo + 2, :, :, :],
                            in_=yq[:],
                        )

                # ---- Phase B: pointwise (fp32r) ----
                GRPB = 2
                for b in range(B_LOC):
                    for ob in range(OBLK):
                        osb = ob_pool.tile([128, H, W], F32, name="osb")
                        for g2 in range(H // HPT // GRPB):
                            ps = psB_pool.tile([128, GRPB, HPT, W], F32, name="psB")
                            for cb in range(CBLK):
                                for i in range(GRPB):
                                    pt = g2 * GRPB + i
                                    rhs = y_tiles[cb][
                                        :, pt * HPT : (pt + 1) * HPT, b, :
                                    ].bitcast(F32R)
                                    nc.tensor.matmul(
                                        ps[:, i, :, :],
                                        pw_sb[:, cb, ob * 128 : (ob + 1) * 128],
                                        rhs,
                                        start=(cb == 0),
                                        stop=(cb == CBLK - 1),
                                    )
                            for i in range(GRPB):
                                pt = g2 * GRPB + i
                                nc.scalar.add(
                                    osb[:, pt * HPT : (pt + 1) * HPT, :],
                                    ps[:, i, :, :],
                                    pwb_sb[:, ob : ob + 1],
                                )
                        nc.sync.dma_start(
                            out=out_d[b, ob * 128 : (ob + 1) * 128, :, :],
                            in_=osb[:],
                        )
    return nc


def build_nc_v4(reps=1):
    """v4: bf16 banded depthwise + fp32r pointwise, DMA-count-optimized.

    - x pre-shuffled on host to [128=(q,h'), NPAIR, B_LOC, WP] bf16:
      ONE resident SBUF load; matmul rhs slices directly from it.
    - bands pre-shuffled to [128, NPAIR, K, 64] bf16, loaded in groups.
    - y scatter: one SBUF->SBUF DMA per pair (both channels), issued on the
      scalar-engine HWDGE queue to parallelize with sync-queue DMAs.
    - output: staged per (b, oblk) into a [128, H, W] tile, single DMA out.
    """
    nc = bass.Bass()
    BF16 = mybir.dt.bfloat16
    F32R = mybir.dt.float32r
    WP = W + 2 * PAD
    x_d = nc.dram_tensor("x", [128, NPAIR, B_LOC, WP], BF16, kind="ExternalInput")
    bands_d = nc.dram_tensor("bands", [128, NPAIR, K, 64], BF16, kind="ExternalInput")
    pw_d = nc.dram_tensor("pw", [CBLK, 128, O], F32R, kind="ExternalInput")
    pwb_d = nc.dram_tensor("pwb", [128, OBLK], F32, kind="ExternalInput")
    out_d = nc.dram_tensor("out", [B_LOC, O, H, W], F32, kind="ExternalOutput")

    with TileContext(nc) as tc:
        with (
            tc.tile_pool(name="consts", bufs=1) as consts,
            tc.tile_pool(name="band", bufs=BAND_BUFS) as band_pool,
            tc.tile_pool(name="y", bufs=1) as y_pool,
            tc.tile_pool(name="yq", bufs=6) as yq_pool,
            tc.tile_pool(name="ob", bufs=2) as ob_pool,
            tc.tile_pool(name="psA", bufs=4, space="PSUM") as psA_pool,
            tc.tile_pool(name="psB", bufs=2, space="PSUM") as psB_pool,
        ):
            pw_sb = consts.tile([128, CBLK, O], F32R)
            for cb in range(CBLK):
                nc.sync.dma_start(out=pw_sb[:, cb, :], in_=pw_d[cb])
            pwb_sb = consts.tile([128, OBLK], F32)
            nc.sync.dma_start(out=pwb_sb[:], in_=pwb_d[:])

            y_tiles = [
                y_pool.tile([128, H, B_LOC, W], F32, name=f"y{cb}")
                for cb in range(CBLK)
            ]

            for _rep in range(reps):
                _build_v4_body(
                    nc, tc, consts, band_pool, yq_pool, ob_pool,
                    psA_pool, psB_pool,
                    x_d, bands_d, out_d, pw_sb, pwb_sb, y_tiles, _rep,
                )
    return nc


def _build_v4_body(
    nc, tc, consts, band_pool, yq_pool, ob_pool, psA_pool, psB_pool,
    x_d, bands_d, out_d, pw_sb, pwb_sb, y_tiles, rep,
):
    BF16 = mybir.dt.bfloat16
    F32R = mybir.dt.float32r
    WP = W + 2 * PAD
    if True:
        if True:
            x_sb = consts.tile(
                [128, NPAIR, B_LOC, WP], BF16, name=f"x_sb{rep}", tag="x_sb"
            )
            nc.sync.dma_start(out=x_sb[:], in_=x_d[:])

            # ---- Phase A: banded depthwise on PE ----
            for g in range(NPAIR // GRP_PAIRS):
                bt = band_pool.tile([128, GRP_PAIRS, K, 64], BF16, name="bt")
                nc.sync.dma_start(
                    out=bt[:], in_=bands_d[:, g * GRP_PAIRS : (g + 1) * GRP_PAIRS]
                )
                for jj in range(GRP_PAIRS):
                    j = g * GRP_PAIRS + jj
                    cb, c_lo = (2 * j) // 128, (2 * j) % 128
                    ps = psA_pool.tile([128, B_LOC, W], F32, name="psA")
                    for dx in range(K):
                        for q in range(2):
                            nc.tensor.matmul(
                                ps[q * 64 : (q + 1) * 64, :, :],
                                bt[q * 64 : (q + 1) * 64, jj, dx, :],
                                x_sb[q * 64 : (q + 1) * 64, j, :, dx : dx + W],
                                start=(dx == 0),
                                stop=(dx == K - 1),
                            )
                    yq = yq_pool.tile([128, B_LOC, W], F32, name="yq")
                    nc.vector.tensor_copy(out=yq[:], in_=ps[:])
                    # single scatter: channels (2j, 2j+1) are adjacent partitions
                    eng = nc.scalar if (SPLIT_SCATTER and j % 2 == 0) else (
                        nc.sync if SPLIT_SCATTER else nc.scalar
                    )
                    eng.dma_start(
                        out=y_tiles[cb][c_lo : c_lo + 2, :, :, :],
                        in_=yq[:],
                    )

            # ---- Phase B: pointwise (fp32r) ----
            GRPB = 2
            for b in range(B_LOC):
                for ob in range(OBLK):
                    osb = ob_pool.tile([128, H, W], F32, name="osb")
                    for g in range(H // HPT // GRPB):
                        ps = psB_pool.tile([128, GRPB, HPT, W], F32, name="psB")
                        for cb in range(CBLK):
                            for i in range(GRPB):
                                pt = g * GRPB + i
                                rhs = y_tiles[cb][
                                    :, pt * HPT : (pt + 1) * HPT, b, :
                                ].bitcast(F32R)
                                nc.tensor.matmul(
                                    ps[:, i, :, :],
                                    pw_sb[:, cb, ob * 128 : (ob + 1) * 128],
                                    rhs,
                                    start=(cb == 0),
                                    stop=(cb == CBLK - 1),
                                )
                        for i in range(GRPB):
                            pt = g * GRPB + i
                            nc.scalar.add(
                                osb[:, pt * HPT : (pt + 1) * HPT, :],
                                ps[:, i, :, :],
                                pwb_sb[:, ob : ob + 1],
                            )
                    nc.sync.dma_start(
                        out=out_d[b, ob * 128 : (ob + 1) * 128, :, :],
                        in_=osb[:],
                    )
    return nc


def build_nc(dw_impl=DW_IMPL):
    reps = int(dw_impl.split("@")[1]) if "@" in dw_impl else 1
    if dw_impl.startswith("v4"):
        return build_nc_v4(reps=reps)
    if dw_impl.startswith("v5"):
        return build_nc_v5(reps=reps)
    nc = bass.Bass()
    BF16 = mybir.dt.bfloat16
    F32R = mybir.dt.float32r
    dt_a = BF16 if dw_impl == "pe16" else F32
    dt_b = F32R if dw_impl == "pe16" else F32
    if dw_impl in ("pe", "pe16"):
        x_d = nc.dram_tensor(
            "x", [B_LOC, C, H, W + 2 * PAD], dt_a, kind="ExternalInput"
        )
        bands_d = nc.dram_tensor(
            "bands", [NPAIR, 128, K, 64], dt_a, kind="ExternalInput"
        )
    else:
        x_d = nc.dram_tensor("x", [B_LOC, C, H, W], F32, kind="ExternalInput")
        dwt_d = nc.dram_tensor("dwt", [128, CBLK, K * K], F32, kind="ExternalInput")
    pw_d = nc.dram_tensor("pw", [CBLK, 128, O], dt_b, kind="ExternalInput")
    pwb_d = nc.dram_tensor("pwb", [128, OBLK], F32, kind="ExternalInput")
    out_d = nc.dram_tensor("out", [B_LOC, O, H, W], F32, kind="ExternalOutput")

    with TileContext(nc) as tc:
        with (
            tc.tile_pool(name="consts", bufs=1) as consts,
            tc.tile_pool(name="xp", bufs=3) as xp_pool,
            tc.tile_pool(name="y", bufs=1) as y_pool,
            tc.tile_pool(name="ob", bufs=3) as ob_pool,
            tc.tile_pool(name="psB", bufs=2, space="PSUM") as psB_pool,
            tc.tile_pool(name="psA", bufs=2, space="PSUM") as psA_pool,
            tc.tile_pool(name="band", bufs=4) as band_pool,
            tc.tile_pool(name="yq", bufs=4) as yq_pool,
        ):
            pools = {
                "xp": xp_pool,
                "xp_pe": xp_pool,
                "band": band_pool,
                "psA": psA_pool,
                "yq": yq_pool,
            }
            pw_sb = consts.tile([128, CBLK, O], dt_b)
            for cb in range(CBLK):
                nc.default_dma_engine.dma_start(out=pw_sb[:, cb, :], in_=pw_d[cb])
            pwb_sb = consts.tile([128, OBLK], F32)
            nc.default_dma_engine.dma_start(out=pwb_sb[:], in_=pwb_d[:])

            if dw_impl in ("pe", "pe16"):
                # y layout: [c 128, h 64, b B_LOC, w 64] per channel block
                y_tiles = [
                    y_pool.tile([128, H, B_LOC, W], F32, name=f"y{cb}")
                    for cb in range(CBLK)
                ]
                _build_pe_phase_a(nc, tc, pools, x_d, bands_d, y_tiles, dt_a=dt_a)
                # Phase B: pointwise, psum groups of 2 pixel-tiles
                GRP = 2
                for b in range(B_LOC):
                    for ob in range(OBLK):
                        for g in range(H // HPT // GRP):
                            ps = psB_pool.tile([128, GRP, HPT, W], F32, name="psB")
                            for cb in range(CBLK):
                                for i in range(GRP):
                                    pt = g * GRP + i
                                    rhs = y_tiles[cb][:, pt * HPT : (pt + 1) * HPT, b, :]
                                    if dt_b != F32:
                                        rhs = rhs.bitcast(dt_b)
                                    nc.tensor.matmul(
                                        ps[:, i, :, :],
                                        pw_sb[:, cb, ob * 128 : (ob + 1) * 128],
                                        rhs,
                                        start=(cb == 0),
                                        stop=(cb == CBLK - 1),
                                    )
                            for i in range(GRP):
                                pt = g * GRP + i
                                osb = ob_pool.tile([128, HPT, W], F32, name="osb")
                                nc.scalar.add(
                                    osb[:], ps[:, i, :, :], pwb_sb[:, ob : ob + 1]
                                )
                                nc.default_dma_engine.dma_start(
                                    out=out_d[b, ob * 128 : (ob + 1) * 128,
                                              pt * HPT : (pt + 1) * HPT, :],
                                    in_=osb[:],
                                )
            else:
                dwt_sb = consts.tile([128, CBLK, K * K], F32)
                nc.default_dma_engine.dma_start(out=dwt_sb[:], in_=dwt_d[:])
                for b in range(B_LOC):
                    y_tiles = [
                        y_pool.tile([128, H, W], F32, tag=f"y{cb}", name=f"y{cb}_{b}")
                        for cb in range(CBLK)
                    ]
                    _build_dve_phase_a(nc, tc, pools, x_d, dwt_sb, y_tiles, b)
                    for ob in range(OBLK):
                        for pt in range(H // HPT):
                            ps = psB_pool.tile([128, HPT, W], F32, name="psB")
                            for cb in range(CBLK):
                                nc.tensor.matmul(
                                    ps[:],
                                    pw_sb[:, cb, ob * 128 : (ob + 1) * 128],
                                    y_tiles[cb][:, pt * HPT : (pt + 1) * HPT, :],
                                    start=(cb == 0),
                                    stop=(cb == CBLK - 1),
                                )
                            osb = ob_pool.tile([128, HPT, W], F32, name="osb")
                            nc.scalar.add(osb[:], ps[:], pwb_sb[:, ob : ob + 1])
                            nc.default_dma_engine.dma_start(
                                out=out_d[b, ob * 128 : (ob + 1) * 128,
                                          pt * HPT : (pt + 1) * HPT, :],
                                in_=osb[:],
                            )
    return nc


def legalize_waits(nc, max_waits=1):
    """This container's walrus accepts only one sync-wait per instruction.

    Hoist extra on_wait conditions into standalone same-engine NoOps placed
    immediately before the instruction (engine programs execute in block
    order, so the waits still complete before the instruction issues).
    """
    n_hoisted = 0
    for f in nc.m.functions:
        for blk in f.blocks:
            insts = list(blk.instructions)
            out = []
            changed = False
            for inst in insts:
                si = inst.sync_info
                if si is not None and si.on_wait and len(si.on_wait) > max_waits:
                    waits = list(si.on_wait)
                    for i, w in enumerate(waits[:-max_waits]):
                        nop = mybir.InstNoOp(name=f"{inst.name}-hw{i}")
                        nop.engine = inst.engine
                        nop.sync_info = mybir.SyncInfo(on_wait=[w], on_update=[])
                        out.append(nop)
                        n_hoisted += 1
                    inst.sync_info = mybir.SyncInfo(
                        on_wait=waits[-max_waits:], on_update=list(si.on_update)
                    )
                    changed = True
                out.append(inst)
            if changed:
                blk.instructions = out
    return n_hoisted


_NC_CACHE = {}


def _get_nc(dw_impl):
    if dw_impl not in _NC_CACHE:
        nc = build_nc(dw_impl)
        legalize_waits(nc)
        _NC_CACHE[dw_impl] = nc
    return _NC_CACHE[dw_impl]


def _build_bands(dw_w):
    """bands[j, q*64+hp, dx, h] = dw_w[2j+q, 0, hp-h+3, dx] (0 outside band)."""
    dw = dw_w[:, 0].reshape(NPAIR, 2, K, K)  # [j, q, dy, dx]
    bands = np.zeros((NPAIR, 2, 64, K, 64), np.float32)
    for dy in range(K):
        for hp in range(64):
            h = hp - dy + PAD
            if 0 <= h < 64:
                bands[:, :, hp, :, h] = dw[:, :, dy, :]
    return np.ascontiguousarray(bands.reshape(NPAIR, 128, K, 64))


def _prep_in_maps(x, dw_w, dw_b, pw_w, pw_b, dw_impl=None):
    """Host-side weight prep + per-core sharding. Returns in_maps list."""
    if dw_impl is None:
        dw_impl = DW_IMPL
    pw_mat = pw_w[:, :, 0, 0].T  # [C, O] (c-major)
    pw = np.ascontiguousarray(pw_mat.reshape(CBLK, 128, O))
    pwb_eff = pw_b + pw_mat.T @ dw_b  # [O]
    pwb = np.ascontiguousarray(pwb_eff.reshape(OBLK, 128).T)  # [128, OBLK]
    if dw_impl.startswith(("v4", "v5")):
        import ml_dtypes

        WP = W + 2 * PAD
        xp = np.zeros((B, C, H, WP), np.float32)
        xp[:, :, :, PAD : PAD + W] = x
        # per-core shard then shuffle to [128=(q,h'), NPAIR, B_LOC, WP]
        # partition p = q*64 + h', where channel c = 2j + q
        bands = _build_bands(dw_w)  # [NPAIR, 128, K, 64]
        if dw_impl.startswith("v5"):
            # padded block-diagonal [128(q,h'), NPAIR, K, 128(q2,h)]
            bp = np.zeros((2, 64, NPAIR, K, 2, 64), np.float32)
            br = bands.reshape(NPAIR, 2, 64, K, 64)
            for q in range(2):
                bp[q, :, :, :, q, :] = br[:, q].transpose(1, 0, 2, 3)
            bands_sh = np.ascontiguousarray(
                bp.reshape(128, NPAIR, K, 128).astype(ml_dtypes.bfloat16)
            )
        else:
            bands_sh = np.ascontiguousarray(
                bands.transpose(1, 0, 2, 3).astype(ml_dtypes.bfloat16)
            )  # [128, NPAIR, K, 64]
        shared = {"bands": bands_sh, "pw": pw, "pwb": pwb}
        in_maps = []
        for k in range(N_CORES):
            xk = xp[k * B_LOC : (k + 1) * B_LOC]  # [B_LOC, C, H, WP]
            # -> [q, h', j, b, wp] -> [(q h'), j, b, wp]
            xr = xk.reshape(B_LOC, NPAIR, 2, H, WP)
            xsh = np.ascontiguousarray(
                xr.transpose(2, 3, 1, 0, 4).reshape(128, NPAIR, B_LOC, WP)
            ).astype(ml_dtypes.bfloat16)
            m = {"x": xsh}
            m.update(shared)
            in_maps.append(m)
        return in_maps
    if dw_impl in ("pe", "pe16"):
        xp = np.zeros((B, C, H, W + 2 * PAD), np.float32)
        xp[:, :, :, PAD : PAD + W] = x
        bands = _build_bands(dw_w)
        if dw_impl == "pe16":
            import ml_dtypes

            xp = xp.astype(ml_dtypes.bfloat16)
            bands = bands.astype(ml_dtypes.bfloat16)
        shared = {"bands": bands, "pw": pw, "pwb": pwb}
        xs = xp
    else:
        dwt = np.ascontiguousarray(
            dw_w[:, 0].reshape(CBLK, 128, K * K).transpose(1, 0, 2)
        )  # [128, CBLK, 49], partition = c_lo
        shared = {"dwt": dwt, "pw": pw, "pwb": pwb}
        xs = x
    in_maps = []
    for k in range(N_CORES):
        m = {"x": np.ascontiguousarray(xs[k * B_LOC : (k + 1) * B_LOC])}
        m.update(shared)
        in_maps.append(m)
    return in_maps


def _make_runner(nc):
    """Compile nc into a pipelined multi-core jitted fn (no donation)."""
    import jax
    from jax.sharding import Mesh, NamedSharding, PartitionSpec
    from jax.experimental.shard_map import shard_map
    from concourse import bass2jax
    from concourse.bass2jax import _bass_exec_p

    bass2jax.install_neuronx_cc_hook()
    n_cores = N_CORES
    partition_name = (
        nc.partition_id_tensor.name if nc.partition_id_tensor else None
    )
    in_names, out_names, out_avals, zero_outs = [], [], [], []
    for alloc in nc.m.functions[0].allocations:
        if not isinstance(alloc, mybir.MemoryLocationSet):
            continue
        name = alloc.memorylocations[0].name
        if alloc.kind == "ExternalInput":
            if name != partition_name:
                in_names.append(name)
        elif alloc.kind == "ExternalOutput":
            out_names.append(name)
            shape = tuple(alloc.tensor_shape)
            dtype = mybir.dt.np(alloc.dtype)
            out_avals.append(jax.core.ShapedArray(shape, dtype))
            zero_outs.append(np.zeros(shape, dtype))
    n_params = len(in_names)
    all_names = in_names + out_names
    if partition_name is not None:
        all_names = all_names + [partition_name]

    def _body(*args):
        operands = list(args)
        if partition_name is not None:
            operands.append(bass2jax.partition_id_tensor())
        outs = _bass_exec_p.bind(
            *operands,
            out_avals=tuple(out_avals),
            in_names=tuple(all_names),
            out_names=tuple(out_names),
            lowering_input_output_aliases=(),
            sim_require_finite=True,
            sim_require_nnan=True,
            nc=nc,
        )
        return tuple(outs)

    devices = jax.devices()[:n_cores]
    mesh = Mesh(np.asarray(devices), ("core",))
    spec = PartitionSpec("core")
    n_all = n_params + len(out_names)
    fn = jax.jit(
        shard_map(
            _body,
            mesh=mesh,
            in_specs=(spec,) * n_all,
            out_specs=(spec,) * len(out_names),
            check_rep=False,
        ),
        keep_unused=True,
    )
    sh = NamedSharding(mesh, spec)
    return fn, in_names, out_names, zero_outs, sh


_FLOOR_CACHE = {}


def _measure_floor(iters):
    """Per-iteration dispatch overhead of a trivial kernel on this session."""
    import time

    import jax

    if "fn" not in _FLOOR_CACHE:
        nc = bass.Bass()
        a_d = nc.dram_tensor("a", [128, 64], F32, kind="ExternalInput")
        o_d = nc.dram_tensor("o", [128, 64], F32, kind="ExternalOutput")
        with TileContext(nc) as tc:
            with tc.tile_pool(name="p", bufs=2) as pool:
                at = pool.tile([128, 64], F32, name="at")
                nc.default_dma_engine.dma_start(out=at[:], in_=a_d[:])
                ot = pool.tile([128, 64], F32, name="ot")
                nc.vector.tensor_copy(out=ot[:], in_=at[:])
                nc.default_dma_engine.dma_start(out=o_d[:], in_=ot[:])
        legalize_waits(nc)
        fn, in_names, out_names, zeros, sh = _make_runner(nc)
        a = jax.device_put(
            np.zeros((N_CORES * 128, 64), np.float32), sh
        )
        z = jax.device_put(np.zeros((N_CORES * 128, 64), np.float32), sh)
        jax.block_until_ready(fn(a, z))
        _FLOOR_CACHE["fn"] = (fn, a, z)
    fn, a, z = _FLOOR_CACHE["fn"]
    t0 = time.perf_counter()
    r = None
    for _ in range(iters):
        r = fn(a, z)
    jax.block_until_ready(r)
    t1 = time.perf_counter()
    return (t1 - t0) / iters


def _bench_impl(impl, in_maps, iters=100):
    """Time one compiled impl; returns (out_arrs_map, raw_per_iter)."""
    import time

    import jax

    nc = _get_nc(impl)
    fn, in_names, out_names, zero_outs, sh = _make_runner(nc)
    concat_in = [
        np.concatenate([np.asarray(in_maps[c][nm]) for c in range(N_CORES)], axis=0)
        for nm in in_names
    ]
    concat_zeros = [
        np.zeros((N_CORES * z.shape[0], *z.shape[1:]), z.dtype) for z in zero_outs
    ]
    dev_in = [jax.device_put(a, sh) for a in concat_in + concat_zeros]
    out_arrs = jax.block_until_ready(fn(*dev_in))
    # time
    best = None
    for _round in range(3):
        t0 = time.perf_counter()
        r = None
        for _ in range(iters):
            r = fn(*dev_in)
        jax.block_until_ready(r)
        t1 = time.perf_counter()
        v = (t1 - t0) / iters
        best = v if best is None else min(best, v)
    out_full = np.asarray(out_arrs[out_names.index("out")])
    out = out_full.reshape(N_CORES, B_LOC, O, H, W).reshape(B, O, H, W)
    return out, best


def bench_reps(x, dw_w, dw_b, pw_w, pw_b, base="v4", reps=5, iters=100):
    """Floor-free timing: (T(reps) - T(1)) / (reps - 1)."""
    in_maps = _prep_in_maps(
        np.ascontiguousarray(np.asarray(x, dtype=np.float32)),
        np.asarray(dw_w, np.float32),
        np.asarray(dw_b, np.float32),
        np.asarray(pw_w, np.float32),
        np.asarray(pw_b, np.float32),
        dw_impl=base,
    )
    out1, t1 = _bench_impl(base, in_maps, iters)
    _, tR = _bench_impl(f"{base}@{reps}", in_maps, iters)
    per_rep = (tR - t1) / (reps - 1)
    return out1, per_rep, t1, tR


def bench(x, dw_w, dw_b, pw_w, pw_b, iters=200):
    """Steady-state timing with floor subtraction.

    Returns (out, marginal_per_iter_s, raw_per_iter_s, floor_s).
    """
    import time

    import jax

    nc = _get_nc(DW_IMPL)
    in_maps = _prep_in_maps(
        np.ascontiguousarray(np.asarray(x, dtype=np.float32)),
        np.asarray(dw_w, np.float32),
        np.asarray(dw_b, np.float32),
        np.asarray(pw_w, np.float32),
        np.asarray(pw_b, np.float32),
    )
    fn, in_names, out_names, zero_outs, sh = _make_runner(nc)
    concat_in = [
        np.concatenate([np.asarray(in_maps[c][nm]) for c in range(N_CORES)], axis=0)
        for nm in in_names
    ]
    concat_zeros = [
        np.zeros((N_CORES * z.shape[0], *z.shape[1:]), z.dtype) for z in zero_outs
    ]
    dev_in = [jax.device_put(a, sh) for a in concat_in + concat_zeros]
    out_arrs = jax.block_until_ready(fn(*dev_in))  # compile + warm

    floor = _measure_floor(iters)
    t0 = time.perf_counter()
    r = None
    for _ in range(iters):
        r = fn(*dev_in)
    jax.block_until_ready(r)
    t1 = time.perf_counter()
    raw = (t1 - t0) / iters
    out_full = np.asarray(out_arrs[out_names.index("out")])
    out = out_full.reshape(N_CORES, B_LOC, O, H, W).reshape(B, O, H, W)
    return out, max(raw - floor, 0.0), raw, floor


def kernel(x, dw_w, dw_b, pw_w, pw_b, trace=False):
    global LAST_EXEC_NS
    in_maps = _prep_in_maps(
        np.ascontiguousarray(np.asarray(x, dtype=np.float32)),
        np.asarray(dw_w, np.float32),
        np.asarray(dw_b, np.float32),
        np.asarray(pw_w, np.float32),
        np.asarray(pw_b, np.float32),
    )
    nc = _get_nc(DW_IMPL)
    res = run_bass_kernel_spmd(nc, in_maps, list(range(N_CORES)), trace=trace)
    LAST_EXEC_NS = res.exec_time_ns
    out = np.concatenate([res.results[k]["out"] for k in range(N_CORES)], axis=0)
    return out


# revision 37
# speedup vs baseline: 2.2454x; 1.1584x over previous
"""Trainium2 Bass kernel for DepthwiseSeparableConv.

Reference computation (NCHW, fp32):
    y   = depthwise_conv7x7(x, dw_w, pad=3, groups=C_IN) + dw_b
    out = pointwise_1x1(y, pw_w) + pw_b

Shapes: x [16, 256, 64, 64], dw_w [256,1,7,7], dw_b [256],
        pw_w [512,256,1,1], pw_b [512]  ->  out [16, 512, 64, 64].

Strategy: data-parallel over batch across 8 NeuronCores (2 batches/core).
Inside each core:
  Phase A (depthwise): channels on partitions, pixels on the free dim.
      Implementations:
        - "dve": 49 scalar_tensor_tensor MACs per 128-channel block (simple).
        - "pe":  banded matmuls on the TensorEngine (fast). Per channel pair
                 (2 channels x 64 H rows = 128 partitions) and kernel column
                 dx, a host-precomputed 64x64 banded matrix applies the conv
                 along H while the W shift comes from clipped column ranges,
                 accumulating all 7 dx into PSUM via has_written semantics.
  Phase B (pointwise): out[o, px] = sum_c pw[c, o] * y[c, px] as dense
      matmuls, contraction over channels (2 blocks of 128), N=512 pixel tiles.
dw_b is folded into an effective pointwise bias on the host (exact, since
the pointwise is linear): pw_b_eff = pw_b + pw_mat.T @ dw_b.
"""

import os
import sys

sys.path.insert(0, "/opt/trn_rl_repo")

import numpy as np

import concourse.bass as bass
import concourse.mybir as mybir
from concourse.tile import TileContext
from concourse.bass_utils import run_bass_kernel_spmd

F32 = mybir.dt.float32

N_CORES = 8
B, C, O, H, W, K, PAD = 16, 256, 512, 64, 64, 7, 3
B_LOC = B // N_CORES          # 2 batches per core
CBLK = C // 128               # 2 channel blocks
OBLK = O // 128               # 4 output-channel blocks
NPAIR = C // 2                # 128 channel pairs (for "pe" depthwise)
PXT = 512                     # pixel-tile (free dim) for pointwise matmuls
HPT = PXT // W                # h-rows per pixel tile (8)

LAST_EXEC_NS = None

# "pe16": bf16 banded depthwise on PE + float32r pointwise (fast)
# "pe":   fp32 banded depthwise on PE + fp32 pointwise
# "dve":  fp32 VectorEngine MAC depthwise (simple reference)
DW_IMPL = os.environ.get("DSC_DW_IMPL", "pe16")


def _clip(lo, hi):
    return max(lo, 0), min(hi, 64)


def _build_dve_phase_a(nc, tc, pools, x_d, dwt_sb, y_tiles, b):
    """Depthwise via 49 per-partition-scalar MACs on DVE, per channel block."""
    xp_pool = pools["xp"]
    for cb in range(CBLK):
        xs = xp_pool.tile([128, H, W], F32)
        nc.default_dma_engine.dma_start(
            out=xs[:], in_=x_d[b, cb * 128 : (cb + 1) * 128, :, :]
        )
        y = y_tiles[cb]
        # center tap first: initializes every output element (full range)
        taps = [(3, 3)] + [
            (dy, dx) for dy in range(K) for dx in range(K) if (dy, dx) != (3, 3)
        ]
        for dy, dx in taps:
            oh0, oh1 = _clip(PAD - dy, H + PAD - dy)
            ow0, ow1 = _clip(PAD - dx, W + PAD - dx)
            ih0, iw0 = oh0 + dy - PAD, ow0 + dx - PAD
            src = xs[:, ih0 : ih0 + (oh1 - oh0), iw0 : iw0 + (ow1 - ow0)]
            dst = y[:, oh0:oh1, ow0:ow1]
            sc = dwt_sb[:, cb, dy * K + dx : dy * K + dx + 1]
            if (dy, dx) == (3, 3):
                nc.vector.tensor_scalar(
                    out=dst, in0=src, scalar1=sc, scalar2=None,
                    op0=mybir.AluOpType.mult,
                )
            else:
                nc.vector.scalar_tensor_tensor(
                    out=dst, in0=src, scalar=sc, in1=dst,
                    op0=mybir.AluOpType.mult, op1=mybir.AluOpType.add,
                )


def _build_pe_phase_a(nc, tc, pools, x_d, bands_d, y_tiles, dt_a=F32):
    """Depthwise via banded matmuls on the TensorEngine.

    Per channel pair j (channels 2j, 2j+1): SBUF tile xp holds
    [partitions = (q in 2, h' in 64), free = (b in B_LOC, wpad in 70)]
    (x is host-padded along W with 3 zeros each side). For dx in 0..6 and
    q in 0..1, a matmul accumulates the H-direction conv into
    ps[(q,h), (b,w)]:
        lhsT = band block [K=64 (h'), M=64 (h)]  (host-precomputed,
               band[h',h] = dw[c, h'-h+3, dx], zero outside the 7-diag)
        rhs  = xp[q rows, :, dx:dx+64]          (W shift via AP offset)
    The 14 matmuls accumulate via PSUM has_written semantics (start=True
    only on the very first, which clears the whole bank). The result pair
    tile is copied PSUM->SBUF (DVE) and DMA-scattered into the
    channel-major y tiles [c, h, b, w].
    """
    xp_pool = pools["xp_pe"]
    band_pool = pools["band"]
    psA_pool = pools["psA"]
    yq_pool = pools["yq"]
    WP = W + 2 * PAD
    for j in range(NPAIR):
        cb, c_lo = (2 * j) // 128, (2 * j) % 128
        bt = band_pool.tile([128, K, 64], dt_a, name="bt")
        nc.default_dma_engine.dma_start(out=bt[:], in_=bands_d[j])
        xp = xp_pool.tile([128, B_LOC, WP], dt_a, name="xp")
        for b in range(B_LOC):
            nc.default_dma_engine.dma_start(
                out=xp[:, b, :], in_=x_d[b, 2 * j : 2 * j + 2, :, :]
            )
        ps = psA_pool.tile([128, B_LOC, W], F32, name="psA")
        for dx in range(K):
            for q in range(2):
                # per-q accumulation groups: the pending-zero clear of a
                # start=True matmul covers only the partitions it writes
                nc.tensor.matmul(
                    ps[q * 64 : (q + 1) * 64, :, :],
                    bt[q * 64 : (q + 1) * 64, dx, :],
                    xp[q * 64 : (q + 1) * 64, :, dx : dx + W],
                    start=(dx == 0),
                    stop=(dx == K - 1),
                )
        yq = yq_pool.tile([128, B_LOC, W], F32, name="yq")
        nc.vector.tensor_copy(out=yq[:], in_=ps[:])
        for q in range(2):
            nc.default_dma_engine.dma_start(
                out=y_tiles[cb][c_lo + q : c_lo + q + 1, :, :, :],
                in_=yq[q * 64 : (q + 1) * 64, :, :],
            )


GRP_PAIRS = int(os.environ.get("DSC_GRP", "16"))  # band-load group size for v4
GRP_PAIRS5 = 8  # band-load group size for v5 (padded bands, 2x bytes)
SPLIT_SCATTER = os.environ.get("DSC_SPLIT_SCATTER", "0") == "1"
BAND_BUFS = int(os.environ.get("DSC_BAND_BUFS", "2"))
BIAS_ON_DVE = os.environ.get("DSC_BIAS_DVE", "0") == "1"


def build_nc_v5(reps=1):
    """v5: like v4 but with padded block-diagonal [128x128] bands.

    One matmul per (pair, dx) — K=128 contraction covers both channels of
    the pair at once (the off-diagonal quadrants are zero), halving the
    PE instruction count and enabling FWL bf16 weight loads. y scatters
    alternate between the two HWDGE queues (sync / scalar).
    """
    nc = bass.Bass()
    BF16 = mybir.dt.bfloat16
    F32R = mybir.dt.float32r
    WP = W + 2 * PAD
    G = GRP_PAIRS5
    x_d = nc.dram_tensor("x", [128, NPAIR, B_LOC, WP], BF16, kind="ExternalInput")
    bands_d = nc.dram_tensor(
        "bands", [128, NPAIR, K, 128], BF16, kind="ExternalInput"
    )
    pw_d = nc.dram_tensor("pw", [CBLK, 128, O], F32R, kind="ExternalInput")
    pwb_d = nc.dram_tensor("pwb", [128, OBLK], F32, kind="ExternalInput")
    out_d = nc.dram_tensor("out", [B_LOC, O, H, W], F32, kind="ExternalOutput")

    with TileContext(nc) as tc:
        with (
            tc.tile_pool(name="consts", bufs=1) as consts,
            tc.tile_pool(name="band", bufs=2) as band_pool,
            tc.tile_pool(name="y", bufs=1) as y_pool,
            tc.tile_pool(name="yq", bufs=6) as yq_pool,
            tc.tile_pool(name="ob", bufs=2) as ob_pool,
            tc.tile_pool(name="psA", bufs=4, space="PSUM") as psA_pool,
            tc.tile_pool(name="psB", bufs=2, space="PSUM") as psB_pool,
        ):
            pw_sb = consts.tile([128, CBLK, O], F32R)
            for cb in range(CBLK):
                nc.sync.dma_start(out=pw_sb[:, cb, :], in_=pw_d[cb])
            pwb_sb = consts.tile([128, OBLK], F32)
            nc.sync.dma_start(out=pwb_sb[:], in_=pwb_d[:])

            y_tiles = [
                y_pool.tile([128, H, B_LOC, W], F32, name=f"y{cb}")
                for cb in range(CBLK)
            ]

            for rep in range(reps):
                x_sb = consts.tile(
                    [128, NPAIR, B_LOC, WP], BF16, name=f"x_sb{rep}", tag="x_sb"
                )
                nc.sync.dma_start(out=x_sb[:], in_=x_d[:])

                # ---- Phase A ----
                for g in range(NPAIR // G):
                    bt = band_pool.tile([128, G, K, 128], BF16, name="bt")
                    nc.sync.dma_start(
                        out=bt[:], in_=bands_d[:, g * G : (g + 1) * G]
                    )
                    for jj in range(G):
                        j = g * G + jj
                        cb, c_lo = (2 * j) // 128, (2 * j) % 128
                        ps = psA_pool.tile([128, B_LOC, W], F32, name="psA")
                        for dx in range(K):
                            nc.tensor.matmul(
                                ps[:],
                                bt[:, jj, dx, :],
                                x_sb[:, j, :, dx : dx + W],
                                start=(dx == 0),
                                stop=(dx == K - 1),
                            )
                        yq = yq_pool.tile([128, B_LOC, W], F32, name="yq")
                        nc.vector.tensor_copy(out=yq[:], in_=ps[:])
                        eng = nc.scalar if (j % 2 == 0) else nc.sync
                        eng.dma_start(
                            out=y_tiles[cb][c_lo : c_lo + 2, :, :, :],
                            in_=yq[:],
                        )

                # ---- Phase B: pointwise (fp32r) ----
                GRPB = 2
                for b in range(B_LOC):
                    for ob in range(OBLK):
                        osb = ob_pool.tile([128, H, W], F32, name="osb")
                        for g2 in range(H // HPT // GRPB):
                            ps = psB_pool.tile([128, GRPB, HPT, W], F32, name="psB")
                            for cb in range(CBLK):
                                for i in range(GRPB):
                                    pt = g2 * GRPB + i
                                    rhs = y_tiles[cb][
                                        :, pt * HPT : (pt + 1) * HPT, b, :
                                    ].bitcast(F32R)
                                    nc.tensor.matmul(
                                        ps[:, i, :, :],
                                        pw_sb[:, cb, ob * 128 : (ob + 1) * 128],
                                        rhs,
                                        start=(cb == 0),
                                        stop=(cb == CBLK - 1),
                                    )
                            for i in range(GRPB):
                                pt = g2 * GRPB + i
                                nc.scalar.add(
                                    osb[:, pt * HPT : (pt + 1) * HPT, :],
                                    ps[:, i, :, :],
                                    pwb_sb[:, ob : ob + 1],
                                )
                        nc.sync.dma_start(
                            out=out_d[b, ob * 128 : (ob + 1) * 128, :, :],
                            in_=osb[:],
                        )
    return nc


def build_nc_v4(reps=1):
    """v4: bf16 banded depthwise + fp32r pointwise, DMA-count-optimized.

    - x pre-shuffled on host to [128=(q,h'), NPAIR, B_LOC, WP] bf16:
      ONE resident SBUF load; matmul rhs slices directly from it.
    - bands pre-shuffled to [128, NPAIR, K, 64] bf16, loaded in groups.
    - y scatter: one SBUF->SBUF DMA per pair (both channels), issued on the
      scalar-engine HWDGE queue to parallelize with sync-queue DMAs.
    - output: staged per (b, oblk) into a [128, H, W] tile, single DMA out.
    """
    nc = bass.Bass()
    BF16 = mybir.dt.bfloat16
    F32R = mybir.dt.float32r
    WP = W + 2 * PAD
    x_d = nc.dram_tensor("x", [128, NPAIR, B_LOC, WP], BF16, kind="ExternalInput")
    bands_d = nc.dram_tensor("bands", [128, NPAIR, K, 64], BF16, kind="ExternalInput")
    pw_d = nc.dram_tensor("pw", [CBLK, 128, O], F32R, kind="ExternalInput")
    pwb_d = nc.dram_tensor("pwb", [128, OBLK], F32, kind="ExternalInput")
    out_d = nc.dram_tensor("out", [B_LOC, O, H, W], F32, kind="ExternalOutput")

    with TileContext(nc) as tc:
        with (
            tc.tile_pool(name="consts", bufs=1) as consts,
            tc.tile_pool(name="band", bufs=BAND_BUFS) as band_pool,
            tc.tile_pool(name="y", bufs=1) as y_pool,
            tc.tile_pool(name="yq", bufs=6) as yq_pool,
            tc.tile_pool(name="ob", bufs=2) as ob_pool,
            tc.tile_pool(name="psA", bufs=4, space="PSUM") as psA_pool,
            tc.tile_pool(name="psB", bufs=2, space="PSUM") as psB_pool,
        ):
            pw_sb = consts.tile([128, CBLK, O], F32R)
            for cb in range(CBLK):
                nc.sync.dma_start(out=pw_sb[:, cb, :], in_=pw_d[cb])
            pwb_sb = consts.tile([128, OBLK], F32)
            nc.sync.dma_start(out=pwb_sb[:], in_=pwb_d[:])

            y_tiles = [
                y_pool.tile([128, H, B_LOC, W], F32, name=f"y{cb}")
                for cb in range(CBLK)
            ]

            for _rep in range(reps):
                _build_v4_body(
                    nc, tc, consts, band_pool, yq_pool, ob_pool,
                    psA_pool, psB_pool,
                    x_d, bands_d, out_d, pw_sb, pwb_sb, y_tiles, _rep,
                )
    return nc


def _build_v4_body(
    nc, tc, consts, band_pool, yq_pool, ob_pool, psA_pool, psB_pool,
    x_d, bands_d, out_d, pw_sb, pwb_sb, y_tiles, rep,
):
    BF16 = mybir.dt.bfloat16
    F32R = mybir.dt.float32r
    WP = W + 2 * PAD
    only = os.environ.get("DSC_ONLY", "")
    if only != "b":
        if True:
            x_sb = consts.tile(
                [128, NPAIR, B_LOC, WP], BF16, name=f"x_sb{rep}", tag="x_sb"
            )
            nc.sync.dma_start(out=x_sb[:], in_=x_d[:])

            # ---- Phase A: banded depthwise on PE ----
            for g in range(NPAIR // GRP_PAIRS):
                bt = band_pool.tile([128, GRP_PAIRS, K, 64], BF16, name="bt")
                nc.sync.dma_start(
                    out=bt[:], in_=bands_d[:, g * GRP_PAIRS : (g + 1) * GRP_PAIRS]
                )
                for jj in range(GRP_PAIRS):
                    j = g * GRP_PAIRS + jj
                    cb, c_lo = (2 * j) // 128, (2 * j) % 128
                    ps = psA_pool.tile([128, B_LOC, W], F32, name="psA")
                    for dx in range(K):
                        for q in range(2):
                            nc.tensor.matmul(
                                ps[q * 64 : (q + 1) * 64, :, :],
                                bt[q * 64 : (q + 1) * 64, jj, dx, :],
                                x_sb[q * 64 : (q + 1) * 64, j, :, dx : dx + W],
                                start=(dx == 0),
                                stop=(dx == K - 1),
                            )
                    yq = yq_pool.tile([128, B_LOC, W], F32, name="yq")
                    nc.vector.tensor_copy(out=yq[:], in_=ps[:])
                    # single scatter: channels (2j, 2j+1) are adjacent partitions
                    eng = nc.scalar if (SPLIT_SCATTER and j % 2 == 0) else (
                        nc.sync if SPLIT_SCATTER else nc.scalar
                    )
                    eng.dma_start(
                        out=y_tiles[cb][c_lo : c_lo + 2, :, :, :],
                        in_=yq[:],
                    )

    # ---- Phase B: pointwise (fp32r) ----
    if only == "a":
        return nc
    GRPB = 2
    for b in range(B_LOC):
        for ob in range(OBLK):
            osb = ob_pool.tile([128, H, W], F32, name="osb")
            for g in range(H // HPT // GRPB):
                ps = psB_pool.tile([128, GRPB, HPT, W], F32, name="psB")
                for cb in range(CBLK):
                    for i in range(GRPB):
                        pt = g * GRPB + i
                        rhs = y_tiles[cb][
                            :, pt * HPT : (pt + 1) * HPT, b, :
                        ].bitcast(F32R)
                        nc.tensor.matmul(
                            ps[:, i, :, :],
                            pw_sb[:, cb, ob * 128 : (ob + 1) * 128],
                            rhs,
                            start=(cb == 0),
                            stop=(cb == CBLK - 1),
                        )
                for i in range(GRPB):
                    pt = g * GRPB + i
                    if BIAS_ON_DVE:
                        nc.vector.tensor_scalar(
                            out=osb[:, pt * HPT : (pt + 1) * HPT, :],
                            in0=ps[:, i, :, :],
                            scalar1=pwb_sb[:, ob : ob + 1],
                            scalar2=None,
                            op0=mybir.AluOpType.add,
                        )
                    else:
                        nc.scalar.add(
                            osb[:, pt * HPT : (pt + 1) * HPT, :],
                            ps[:, i, :, :],
                            pwb_sb[:, ob : ob + 1],
                        )
            nc.sync.dma_start(
                out=out_d[b, ob * 128 : (ob + 1) * 128, :, :],
                in_=osb[:],
            )
    return nc


def build_nc(dw_impl=DW_IMPL):
    reps = int(dw_impl.split("@")[1]) if "@" in dw_impl else 1
    if dw_impl.startswith("v4"):
        return build_nc_v4(reps=reps)
    if dw_impl.startswith("v5"):
        return build_nc_v5(reps=reps)
    nc = bass.Bass()
    BF16 = mybir.dt.bfloat16
    F32R = mybir.dt.float32r
    dt_a = BF16 if dw_impl == "pe16" else F32
    dt_b = F32R if dw_impl == "pe16" else F32
    if dw_impl in ("pe", "pe16"):
        x_d = nc.dram_tensor(
            "x", [B_LOC, C, H, W + 2 * PAD], dt_a, kind="ExternalInput"
        )
        bands_d = nc.dram_tensor(
            "bands", [NPAIR, 128, K, 64], dt_a, kind="ExternalInput"
        )
    else:
        x_d = nc.dram_tensor("x", [B_LOC, C, H, W], F32, kind="ExternalInput")
        dwt_d = nc.dram_tensor("dwt", [128, CBLK, K * K], F32, kind="ExternalInput")
    pw_d = nc.dram_tensor("pw", [CBLK, 128, O], dt_b, kind="ExternalInput")
    pwb_d = nc.dram_tensor("pwb", [128, OBLK], F32, kind="ExternalInput")
    out_d = nc.dram_tensor("out", [B_LOC, O, H, W], F32, kind="ExternalOutput")

    with TileContext(nc) as tc:
        with (
            tc.tile_pool(name="consts", bufs=1) as consts,
            tc.tile_pool(name="xp", bufs=3) as xp_pool,
            tc.tile_pool(name="y", bufs=1) as y_pool,
            tc.tile_pool(name="ob", bufs=3) as ob_pool,
            tc.tile_pool(name="psB", bufs=2, space="PSUM") as psB_pool,
            tc.tile_pool(name="psA", bufs=2, space="PSUM") as psA_pool,
            tc.tile_pool(name="band", bufs=4) as band_pool,
            tc.tile_pool(name="yq", bufs=4) as yq_pool,
        ):
            pools = {
                "xp": xp_pool,
                "xp_pe": xp_pool,
                "band": band_pool,
                "psA": psA_pool,
                "yq": yq_pool,
            }
            pw_sb = consts.tile([128, CBLK, O], dt_b)
            for cb in range(CBLK):
                nc.default_dma_engine.dma_start(out=pw_sb[:, cb, :], in_=pw_d[cb])
            pwb_sb = consts.tile([128, OBLK], F32)
            nc.default_dma_engine.dma_start(out=pwb_sb[:], in_=pwb_d[:])

            if dw_impl in ("pe", "pe16"):
                # y layout: [c 128, h 64, b B_LOC, w 64] per channel block
                y_tiles = [
                    y_pool.tile([128, H, B_LOC, W], F32, name=f"y{cb}")
                    for cb in range(CBLK)
                ]
                _build_pe_phase_a(nc, tc, pools, x_d, bands_d, y_tiles, dt_a=dt_a)
                # Phase B: pointwise, psum groups of 2 pixel-tiles
                GRP = 2
                for b in range(B_LOC):
                    for ob in range(OBLK):
                        for g in range(H // HPT // GRP):
                            ps = psB_pool.tile([128, GRP, HPT, W], F32, name="psB")
                            for cb in range(CBLK):
                                for i in range(GRP):
                                    pt = g * GRP + i
                                    rhs = y_tiles[cb][:, pt * HPT : (pt + 1) * HPT, b, :]
                                    if dt_b != F32:
                                        rhs = rhs.bitcast(dt_b)
                                    nc.tensor.matmul(
                                        ps[:, i, :, :],
                                        pw_sb[:, cb, ob * 128 : (ob + 1) * 128],
                                        rhs,
                                        start=(cb == 0),
                                        stop=(cb == CBLK - 1),
                                    )
                            for i in range(GRP):
                                pt = g * GRP + i
                                osb = ob_pool.tile([128, HPT, W], F32, name="osb")
                                nc.scalar.add(
                                    osb[:], ps[:, i, :, :], pwb_sb[:, ob : ob + 1]
                                )
                                nc.default_dma_engine.dma_start(
                                    out=out_d[b, ob * 128 : (ob + 1) * 128,
                                              pt * HPT : (pt + 1) * HPT, :],
                                    in_=osb[:],
                                )
            else:
                dwt_sb = consts.tile([128, CBLK, K * K], F32)
                nc.default_dma_engine.dma_start(out=dwt_sb[:], in_=dwt_d[:])
                for b in range(B_LOC):
                    y_tiles = [
                        y_pool.tile([128, H, W], F32, tag=f"y{cb}", name=f"y{cb}_{b}")
                        for cb in range(CBLK)
                    ]
                    _build_dve_phase_a(nc, tc, pools, x_d, dwt_sb, y_tiles, b)
                    for ob in range(OBLK):
                        for pt in range(H // HPT):
                            ps = psB_pool.tile([128, HPT, W], F32, name="psB")
                            for cb in range(CBLK):
                                nc.tensor.matmul(
                                    ps[:],
                                    pw_sb[:, cb, ob * 128 : (ob + 1) * 128],
                                    y_tiles[cb][:, pt * HPT : (pt + 1) * HPT, :],
                                    start=(cb == 0),
                                    stop=(cb == CBLK - 1),
                                )
                            osb = ob_pool.tile([128, HPT, W], F32, name="osb")
                            nc.scalar.add(osb[:], ps[:], pwb_sb[:, ob : ob + 1])
                            nc.default_dma_engine.dma_start(
                                out=out_d[b, ob * 128 : (ob + 1) * 128,
                                          pt * HPT : (pt + 1) * HPT, :],
                                in_=osb[:],
                            )
    return nc


def legalize_waits(nc, max_waits=1):
    """This container's walrus accepts only one sync-wait per instruction.

    Hoist extra on_wait conditions into standalone same-engine NoOps placed
    immediately before the instruction (engine programs execute in block
    order, so the waits still complete before the instruction issues).
    """
    n_hoisted = 0
    for f in nc.m.functions:
        for blk in f.blocks:
            insts = list(blk.instructions)
            out = []
            changed = False
            for inst in insts:
                si = inst.sync_info
                if si is not None and si.on_wait and len(si.on_wait) > max_waits:
                    waits = list(si.on_wait)
                    for i, w in enumerate(waits[:-max_waits]):
                        nop = mybir.InstNoOp(name=f"{inst.name}-hw{i}")
                        nop.engine = inst.engine
                        nop.sync_info = mybir.SyncInfo(on_wait=[w], on_update=[])
                        out.append(nop)
                        n_hoisted += 1
                    inst.sync_info = mybir.SyncInfo(
                        on_wait=waits[-max_waits:], on_update=list(si.on_update)
                    )
                    changed = True
                out.append(inst)
            if changed:
                blk.instructions = out
    return n_hoisted


_NC_CACHE = {}


def _get_nc(dw_impl):
    if dw_impl not in _NC_CACHE:
        nc = build_nc(dw_impl)
        legalize_waits(nc)
        _NC_CACHE[dw_impl] = nc
    return _NC_CACHE[dw_impl]


def _build_bands(dw_w):
    """bands[j, q*64+hp, dx, h] = dw_w[2j+q, 0, hp-h+3, dx] (0 outside band)."""
    dw = dw_w[:, 0].reshape(NPAIR, 2, K, K)  # [j, q, dy, dx]
    bands = np.zeros((NPAIR, 2, 64, K, 64), np.float32)
    for dy in range(K):
        for hp in range(64):
            h = hp - dy + PAD
            if 0 <= h < 64:
                bands[:, :, hp, :, h] = dw[:, :, dy, :]
    return np.ascontiguousarray(bands.reshape(NPAIR, 128, K, 64))


def _prep_in_maps(x, dw_w, dw_b, pw_w, pw_b, dw_impl=None):
    """Host-side weight prep + per-core sharding. Returns in_maps list."""
    if dw_impl is None:
        dw_impl = DW_IMPL
    pw_mat = pw_w[:, :, 0, 0].T  # [C, O] (c-major)
    pw = np.ascontiguousarray(pw_mat.reshape(CBLK, 128, O))
    pwb_eff = pw_b + pw_mat.T @ dw_b  # [O]
    pwb = np.ascontiguousarray(pwb_eff.reshape(OBLK, 128).T)  # [128, OBLK]
    if dw_impl.startswith(("v4", "v5")):
        import ml_dtypes

        WP = W + 2 * PAD
        xp = np.zeros((B, C, H, WP), np.float32)
        xp[:, :, :, PAD : PAD + W] = x
        # per-core shard then shuffle to [128=(q,h'), NPAIR, B_LOC, WP]
        # partition p = q*64 + h', where channel c = 2j + q
        bands = _build_bands(dw_w)  # [NPAIR, 128, K, 64]
        if dw_impl.startswith("v5"):
            # padded block-diagonal [128(q,h'), NPAIR, K, 128(q2,h)]
            bp = np.zeros((2, 64, NPAIR, K, 2, 64), np.float32)
            br = bands.reshape(NPAIR, 2, 64, K, 64)
            for q in range(2):
                bp[q, :, :, :, q, :] = br[:, q].transpose(1, 0, 2, 3)
            bands_sh = np.ascontiguousarray(
                bp.reshape(128, NPAIR, K, 128).astype(ml_dtypes.bfloat16)
            )
        else:
            bands_sh = np.ascontiguousarray(
                bands.transpose(1, 0, 2, 3).astype(ml_dtypes.bfloat16)
            )  # [128, NPAIR, K, 64]
        shared = {"bands": bands_sh, "pw": pw, "pwb": pwb}
        in_maps = []
        for k in range(N_CORES):
            xk = xp[k * B_LOC : (k + 1) * B_LOC]  # [B_LOC, C, H, WP]
            # -> [q, h', j, b, wp] -> [(q h'), j, b, wp]
            xr = xk.reshape(B_LOC, NPAIR, 2, H, WP)
            xsh = np.ascontiguousarray(
                xr.transpose(2, 3, 1, 0, 4).reshape(128, NPAIR, B_LOC, WP)
            ).astype(ml_dtypes.bfloat16)
            m = {"x": xsh}
            m.update(shared)
            in_maps.append(m)
        return in_maps
    if dw_impl in ("pe", "pe16"):
        xp = np.zeros((B, C, H, W + 2 * PAD), np.float32)
        xp[:, :, :, PAD : PAD + W] = x
        bands = _build_bands(dw_w)
        if dw_impl == "pe16":
            import ml_dtypes

            xp = xp.astype(ml_dtypes.bfloat16)
            bands = bands.astype(ml_dtypes.bfloat16)
        shared = {"bands": bands, "pw": pw, "pwb": pwb}
        xs = xp
    else:
        dwt = np.ascontiguousarray(
            dw_w[:, 0].reshape(CBLK, 128, K * K).transpose(1, 0, 2)
        )  # [128, CBLK, 49], partition = c_lo
        shared = {"dwt": dwt, "pw": pw, "pwb": pwb}
        xs = x
    in_maps = []
    for k in range(N_CORES):
        m = {"x": np.ascontiguousarray(xs[k * B_LOC : (k + 1) * B_LOC])}
        m.update(shared)
        in_maps.append(m)
    return in_maps


def _make_runner(nc):
    """Compile nc into a pipelined multi-core jitted fn (no donation)."""
    import jax
    from jax.sharding import Mesh, NamedSharding, PartitionSpec
    from jax.experimental.shard_map import shard_map
    from concourse import bass2jax
    from concourse.bass2jax import _bass_exec_p

    bass2jax.install_neuronx_cc_hook()
    n_cores = N_CORES
    partition_name = (
        nc.partition_id_tensor.name if nc.partition_id_tensor else None
    )
    in_names, out_names, out_avals, zero_outs = [], [], [], []
    for alloc in nc.m.functions[0].allocations:
        if not isinstance(alloc, mybir.MemoryLocationSet):
            continue
        name = alloc.memorylocations[0].name
        if alloc.kind == "ExternalInput":
            if name != partition_name:
                in_names.append(name)
        elif alloc.kind == "ExternalOutput":
            out_names.append(name)
            shape = tuple(alloc.tensor_shape)
            dtype = mybir.dt.np(alloc.dtype)
            out_avals.append(jax.core.ShapedArray(shape, dtype))
            zero_outs.append(np.zeros(shape, dtype))
    n_params = len(in_names)
    all_names = in_names + out_names
    if partition_name is not None:
        all_names = all_names + [partition_name]

    def _body(*args):
        operands = list(args)
        if partition_name is not None:
            operands.append(bass2jax.partition_id_tensor())
        outs = _bass_exec_p.bind(
            *operands,
            out_avals=tuple(out_avals),
            in_names=tuple(all_names),
            out_names=tuple(out_names),
            lowering_input_output_aliases=(),
            sim_require_finite=True,
            sim_require_nnan=True,
            nc=nc,
        )
        return tuple(outs)

    devices = jax.devices()[:n_cores]
    mesh = Mesh(np.asarray(devices), ("core",))
    spec = PartitionSpec("core")
    n_all = n_params + len(out_names)
    fn = jax.jit(
        shard_map(
            _body,
            mesh=mesh,
            in_specs=(spec,) * n_all,
            out_specs=(spec,) * len(out_names),
            check_rep=False,
        ),
        keep_unused=True,
    )
    sh = NamedSharding(mesh, spec)
    return fn, in_names, out_names, zero_outs, sh


_FLOOR_CACHE = {}


def _measure_floor(iters):
    """Per-iteration dispatch overhead of a trivial kernel on this session."""
    import time

    import jax

    if "fn" not in _FLOOR_CACHE:
        nc = bass.Bass()
        a_d = nc.dram_tensor("a", [128, 64], F32, kind="ExternalInput")
        o_d = nc.dram_tensor("o", [128, 64], F32, kind="ExternalOutput")
        with TileContext(nc) as tc:
            with tc.tile_pool(name="p", bufs=2) as pool:
                at = pool.tile([128, 64], F32, name="at")
                nc.default_dma_engine.dma_start(out=at[:], in_=a_d[:])
                ot = pool.tile([128, 64], F32, name="ot")
                nc.vector.tensor_copy(out=ot[:], in_=at[:])
                nc.default_dma_engine.dma_start(out=o_d[:], in_=ot[:])
        legalize_waits(nc)
        fn, in_names, out_names, zeros, sh = _make_runner(nc)
        a = jax.device_put(
            np.zeros((N_CORES * 128, 64), np.float32), sh
        )
        z = jax.device_put(np.zeros((N_CORES * 128, 64), np.float32), sh)
        jax.block_until_ready(fn(a, z))
        _FLOOR_CACHE["fn"] = (fn, a, z)
    fn, a, z = _FLOOR_CACHE["fn"]
    t0 = time.perf_counter()
    r = None
    for _ in range(iters):
        r = fn(a, z)
    jax.block_until_ready(r)
    t1 = time.perf_counter()
    return (t1 - t0) / iters


def _bench_impl(impl, in_maps, iters=100):
    """Time one compiled impl; returns (out_arrs_map, raw_per_iter)."""
    import time

    import jax

    nc = _get_nc(impl)
    fn, in_names, out_names, zero_outs, sh = _make_runner(nc)
    concat_in = [
        np.concatenate([np.asarray(in_maps[c][nm]) for c in range(N_CORES)], axis=0)
        for nm in in_names
    ]
    concat_zeros = [
        np.zeros((N_CORES * z.shape[0], *z.shape[1:]), z.dtype) for z in zero_outs
    ]
    dev_in = [jax.device_put(a, sh) for a in concat_in + concat_zeros]
    out_arrs = jax.block_until_ready(fn(*dev_in))
    # time
    best = None
    for _round in range(3):
        t0 = time.perf_counter()
        r = None
        for _ in range(iters):
            r = fn(*dev_in)
        jax.block_until_ready(r)
        t1 = time.perf_counter()
        v = (t1 - t0) / iters
        best = v if best is None else min(best, v)
    out_full = np.asarray(out_arrs[out_names.index("out")])
    out = out_full.reshape(N_CORES, B_LOC, O, H, W).reshape(B, O, H, W)
    return out, best


def bench_reps(x, dw_w, dw_b, pw_w, pw_b, base="v4", reps=5, iters=100):
    """Floor-free timing: (T(reps) - T(1)) / (reps - 1)."""
    in_maps = _prep_in_maps(
        np.ascontiguousarray(np.asarray(x, dtype=np.float32)),
        np.asarray(dw_w, np.float32),
        np.asarray(dw_b, np.float32),
        np.asarray(pw_w, np.float32),
        np.asarray(pw_b, np.float32),
        dw_impl=base,
    )
    out1, t1 = _bench_impl(base, in_maps, iters)
    _, tR = _bench_impl(f"{base}@{reps}", in_maps, iters)
    per_rep = (tR - t1) / (reps - 1)
    return out1, per_rep, t1, tR


def bench(x, dw_w, dw_b, pw_w, pw_b, iters=200):
    """Steady-state timing with floor subtraction.

    Returns (out, marginal_per_iter_s, raw_per_iter_s, floor_s).
    """
    import time

    import jax

    nc = _get_nc(DW_IMPL)
    in_maps = _prep_in_maps(
        np.ascontiguousarray(np.asarray(x, dtype=np.float32)),
        np.asarray(dw_w, np.float32),
        np.asarray(dw_b, np.float32),
        np.asarray(pw_w, np.float32),
        np.asarray(pw_b, np.float32),
    )
    fn, in_names, out_names, zero_outs, sh = _make_runner(nc)
    concat_in = [
        np.concatenate([np.asarray(in_maps[c][nm]) for c in range(N_CORES)], axis=0)
        for nm in in_names
    ]
    concat_zeros = [
        np.zeros((N_CORES * z.shape[0], *z.shape[1:]), z.dtype) for z in zero_outs
    ]
    dev_in = [jax.device_put(a, sh) for a in concat_in + concat_zeros]
    out_arrs = jax.block_until_ready(fn(*dev_in))  # compile + warm

    floor = _measure_floor(iters)
    t0 = time.perf_counter()
    r = None
    for _ in range(iters):
        r = fn(*dev_in)
    jax.block_until_ready(r)
    t1 = time.perf_counter()
    raw = (t1 - t0) / iters
    out_full = np.asarray(out_arrs[out_names.index("out")])
    out = out_full.reshape(N_CORES, B_LOC, O, H, W).reshape(B, O, H, W)
    return out, max(raw - floor, 0.0), raw, floor


def kernel(x, dw_w, dw_b, pw_w, pw_b, trace=False):
    global LAST_EXEC_NS
    in_maps = _prep_in_maps(
        np.ascontiguousarray(np.asarray(x, dtype=np.float32)),
        np.asarray(dw_w, np.float32),
        np.asarray(dw_b, np.float32),
        np.asarray(pw_w, np.float32),
        np.asarray(pw_b, np.float32),
    )
    nc = _get_nc(DW_IMPL)
    res = run_bass_kernel_spmd(nc, in_maps, list(range(N_CORES)), trace=trace)
    LAST_EXEC_NS = res.exec_time_ns
    out = np.concatenate([res.results[k]["out"] for k in range(N_CORES)], axis=0)
    return out
